# revision 16
# baseline (speedup 1.0000x reference)
"""Trainium2 Bass kernel for nn_Attention2d.

Computation: GroupNorm(32 groups) -> 1x1 qkv conv -> 4-head attention over
H*W=4096 positions -> 1x1 proj conv -> residual add.

Sharding: one (batch, head) pair per NeuronCore (B=2 x NH=4 = 8 cores).
Each core:
  - GroupNorm of its batch slice (replicated across the 4 cores of a batch)
  - its head's q/k (with bias) and v^T (no bias)
  - S^T = k^T q in [keys-on-partitions, queries-on-free] layout
    (no max-subtraction: |S/8| <~ 6 so exp is safe in fp32)
  - P^T = exp(S^T/8); PV via matmul with lhsT = [v^T | ones]  -> the ones
    column yields the softmax denominators for free (row 64 of the output)
  - proj partial = Wp[:, head]^T @ PV_raw  (un-normalized)
Host: out[b] = x[b] + proj_bias + sum_h (partial_h / denom_h + Wp_h @ bv_h)
(per-query softmax normalization and the v-bias term commute through proj).

PE-array packing: the S matmuls contract over only hd=64 partitions, so q and
k are duplicated onto partitions 64..127 (by col-packed qkv matmuls that cost
no extra PE time) and each S^T tile is computed as two concurrent matmuls on
row-groups (0,0) and (64,0) -- 2x effective S throughput.

Matmul dtypes: qkv/attention/proj matmuls use float16 operands (1 cycle/row,
fast weight loads, 11-bit mantissa); GroupNorm matmuls and the softmax
denominators stay exact fp32.
"""

import numpy as np

B, C, H, W = 2, 256, 64, 64
HW = H * W           # 4096
GROUPS = 32
NH = 4
HD = C // NH         # 64
EPS = 1e-5
P = 128
IB = 1024            # query block (PSUM-sized)
NIB = HW // IB       # 4
NJC = HW // P        # 32 key chunks
NCORES = B * NH

# "f32": exact fp32 everywhere (slow). "f32r": float32r qkv+attention
# (TF32-like rounding, ~3 cycles/row on HW). "f16": float16 operands.
MM_MODE = "f16"

_module_cache = {}


def _build_module(mm=MM_MODE):
    import concourse.bacc as bacc
    import concourse.tile as tile
    import concourse.mybir as mybir

    dt = mybir.dt
    f32 = dt.float32
    AF = mybir.ActivationFunctionType
    OP = mybir.AluOpType
    if mm == "f32":
        wdt, adt = f32, f32
    elif mm == "f32r":
        wdt, adt = dt.float32r, dt.float32r
    elif mm == "f16":
        wdt, adt = dt.float16, dt.float16
    else:
        raise ValueError(mm)
    pdt = dt.float16 if mm == "f16" else f32

    nc = bacc.Bacc(trn_type="TRN2", target_bir_lowering=False, debug=False)

    # ---- DRAM I/O (per-core tensors; host prepares layouts) ----
    # channel layout everywhere: c = po*128 + pi  ->  [pi, po, ...]
    x_d = nc.dram_tensor("x", [P, 2, HW], f32, kind="ExternalInput").ap()
    wq_d = nc.dram_tensor("wq", [P, 2, HD], wdt, kind="ExternalInput").ap()
    wk_d = nc.dram_tensor("wk", [P, 2, HD], wdt, kind="ExternalInput").ap()
    wv_d = nc.dram_tensor("wv", [P, 2, HD], wdt, kind="ExternalInput").ap()
    wp_d = nc.dram_tensor("wp", [HD, C], pdt, kind="ExternalInput").ap()
    # q/k biases duplicated on both partition halves: [bq; bq]
    bq_d = nc.dram_tensor("bq", [P, 1], f32, kind="ExternalInput").ap()
    bk_d = nc.dram_tensor("bk", [P, 1], f32, kind="ExternalInput").ap()
    gnw_d = nc.dram_tensor("gnw", [P, 2, 1], f32, kind="ExternalInput").ap()
    gnb_d = nc.dram_tensor("gnb", [P, 2, 1], f32, kind="ExternalInput").ap()
    gmat_d = nc.dram_tensor("gmat", [P, 2, GROUPS], f32, kind="ExternalInput").ap()
    gbc_d = nc.dram_tensor("gbc", [GROUPS, 2, P], f32, kind="ExternalInput").ap()
    out_d = nc.dram_tensor("out", [P, 2, HW], f32, kind="ExternalOutput").ap()
    den_d = nc.dram_tensor("den", [NIB, IB], f32, kind="ExternalOutput").ap()

    with tile.TileContext(nc) as tc:
        with (
            tc.tile_pool(name="const", bufs=1) as const,
            tc.tile_pool(name="big", bufs=1) as big,
            tc.tile_pool(name="tmp", bufs=3) as tmp,
            tc.tile_pool(name="pt", bufs=4) as ptp,
            tc.tile_pool(name="oh", bufs=2) as ohp,
            tc.tile_pool(name="ostage", bufs=3) as ostage,
            tc.tile_pool(name="ps_st", bufs=2, space="PSUM") as ps_st,
            tc.tile_pool(name="ps_pv", bufs=1, space="PSUM") as ps_pv,
            tc.tile_pool(name="ps_sm", bufs=2, space="PSUM") as ps_sm,
        ):
            eps_sb = const.tile([GROUPS, 1], f32)
            nc.vector.memset(eps_sb, EPS)
            ones_sb = const.tile([P, 1], f32)
            nc.vector.memset(ones_sb, 1.0)
            # Touch Ln and Exp immediately so walrus's ACT table loads run
            # during the DMA-in phase instead of on the GN critical path.
            warm_sb = tmp.tile([GROUPS, 1], f32, tag="warm")
            nc.scalar.activation(out=warm_sb, in_=eps_sb, func=AF.Ln,
                                 bias=eps_sb, scale=1.0)
            nc.scalar.activation(out=warm_sb, in_=eps_sb, func=AF.Exp, scale=1.0)

            # ---- load everything (x chunked so stats can start early) ----
            wq_sb = const.tile([P, 2, HD], wdt)
            nc.sync.dma_start(wq_sb, wq_d)
            wk_sb = const.tile([P, 2, HD], wdt)
            nc.sync.dma_start(wk_sb, wk_d)
            wv_sb = const.tile([P, 2, HD], wdt)
            nc.sync.dma_start(wv_sb, wv_d)
            wp_sb = const.tile([HD, C], pdt)
            nc.sync.dma_start(wp_sb, wp_d)
            bq_sb = const.tile([P, 1], f32)
            nc.sync.dma_start(bq_sb, bq_d)
            bk_sb = const.tile([P, 1], f32)
            nc.sync.dma_start(bk_sb, bk_d)
            gnw_sb = const.tile([P, 2, 1], f32)
            nc.sync.dma_start(gnw_sb, gnw_d)
            gnb_sb = const.tile([P, 2, 1], f32)
            nc.sync.dma_start(gnb_sb, gnb_d)
            gmat_sb = const.tile([P, 2, GROUPS], f32)
            nc.sync.dma_start(gmat_sb, gmat_d)
            gbc_sb = const.tile([GROUPS, 2, P], f32)
            nc.sync.dma_start(gbc_sb, gbc_d)

            # ---- GroupNorm statistics, overlapped with the x DMA ----
            x_sb = big.tile([P, 2, HW], f32)
            stats = [tmp.tile([P, 8, 6], f32, tag=f"bnstats{po}",
                              name=f"stats{po}")
                     for po in range(2)]
            for c in range(4):
                cs = slice(c * IB, (c + 1) * IB)
                for po in range(2):
                    nc.sync.dma_start(x_sb[:, po, cs], x_d[:, po, cs])
                    for s in (2 * c, 2 * c + 1):
                        nc.vector.bn_stats(
                            out=stats[po][:, s, :],
                            in_=x_sb[:, po, s * 512:(s + 1) * 512],
                        )
            mv = []
            for po in range(2):
                mvp = const.tile([P, 2], f32, tag=f"mv{po}")
                nc.vector.bn_aggr(out=mvp, in_=stats[po])
                msq = tmp.tile([P, 1], f32, tag="msq")
                nc.vector.tensor_mul(msq, mvp[:, 0:1], mvp[:, 0:1])
                nc.vector.tensor_add(mvp[:, 1:2], mvp[:, 1:2], msq)
                mv.append(mvp)

            # group-level [mean_g, E[x^2]_g] via indicator matmul (values 1/8)
            gst_ps = ps_sm.tile([GROUPS, 2], f32, tag="small")
            nc.tensor.matmul(gst_ps, lhsT=gmat_sb[:, 0, :], rhs=mv[0],
                             start=True, stop=False)
            nc.tensor.matmul(gst_ps, lhsT=gmat_sb[:, 1, :], rhs=mv[1],
                             start=False, stop=True)
            gst = const.tile([GROUPS, 2], f32)
            nc.vector.tensor_copy(gst, gst_ps)

            # var_g = E[x^2]_g - mean_g^2 ; rs = rsqrt(var+eps) via exp(-0.5*ln)
            varg = tmp.tile([GROUPS, 1], f32, tag="varg")
            nc.vector.tensor_mul(varg, gst[:, 0:1], gst[:, 0:1])
            nc.vector.tensor_sub(varg, gst[:, 1:2], varg)
            lnv = tmp.tile([GROUPS, 1], f32, tag="lnv")
            nc.scalar.activation(out=lnv, in_=varg, func=AF.Ln,
                                 bias=eps_sb, scale=1.0)
            st = const.tile([GROUPS, 2], f32)  # [rs_g, -mu_g*rs_g]
            nc.scalar.activation(out=st[:, 0:1], in_=lnv, func=AF.Exp, scale=-0.5)
            nc.vector.tensor_mul(st[:, 1:2], gst[:, 0:1], st[:, 0:1])
            nc.vector.tensor_scalar_mul(st[:, 1:2], st[:, 1:2], -1.0)

            # broadcast to channels, apply gn weight/bias; xn = s*x + t
            xn_sb = big.tile([P, 2, HW], wdt)
            sca = []
            for po in range(2):
                stc_ps = ps_sm.tile([P, 2], f32, tag="small")
                nc.tensor.matmul(stc_ps, lhsT=gbc_sb[:, po, :], rhs=st,
                                 start=True, stop=True)
                sc = const.tile([P, 2], f32, tag=f"sca{po}")
                nc.vector.tensor_mul(sc[:, 0:1], stc_ps[:, 0:1], gnw_sb[:, po, :])
                nc.vector.tensor_mul(sc[:, 1:2], stc_ps[:, 1:2], gnw_sb[:, po, :])
                nc.vector.tensor_add(sc[:, 1:2], sc[:, 1:2], gnb_sb[:, po, :])
                sca.append(sc)
            for c in range(4):
                cs = slice(c * IB, (c + 1) * IB)
                for po in range(2):
                    nc.vector.tensor_scalar(
                        out=xn_sb[:, po, cs], in0=x_sb[:, po, cs],
                        scalar1=sca[po][:, 0:1], scalar2=sca[po][:, 1:2],
                        op0=OP.mult, op1=OP.add,
                    )

            # ---- qkv; q and k duplicated onto partitions 64..127 via
            # col-packed matmuls (concurrent on the PE array) ----
            qq_sb = big.tile([P, HW], adt)
            kk_sb = big.tile([P, HW], adt)
            vt_sb = big.tile([P, NJC, HD + 1], adt)
            nc.vector.tensor_copy(
                vt_sb[:, :, HD:HD + 1],
                ones_sb[:, None, :].to_broadcast([P, NJC, 1]),
            )
            for n in range(HW // 512):
                ns = slice(n * 512, (n + 1) * 512)
                for (wsb, bsb, dst) in ((wq_sb, bq_sb, qq_sb),
                                        (wk_sb, bk_sb, kk_sb)):
                    qp = ps_sm.tile([P, 512], f32, tag="small", name="qp")
                    for half in range(2):
                        hs = slice(half * HD, (half + 1) * HD)
                        nc.tensor.matmul(qp[hs, :], lhsT=wsb[:, 0, :],
                                         rhs=xn_sb[:, 0, ns],
                                         start=True, stop=False)
                        nc.tensor.matmul(qp[hs, :], lhsT=wsb[:, 1, :],
                                         rhs=xn_sb[:, 1, ns],
                                         start=False, stop=True)
                    nc.vector.tensor_scalar_add(dst[:, ns], qp, bsb)
            # v^T directly: [positions, head_dim], chunked by 128 positions
            for jc in range(NJC):
                js = slice(jc * P, (jc + 1) * P)
                vp = ps_sm.tile([P, HD], f32, tag="small", name="vp")
                nc.tensor.matmul(vp, lhsT=xn_sb[:, 0, js],
                                 rhs=wv_sb[:, 0, :], start=True, stop=False)
                nc.tensor.matmul(vp, lhsT=xn_sb[:, 1, js],
                                 rhs=wv_sb[:, 1, :], start=False, stop=True)
                nc.vector.tensor_copy(vt_sb[:, jc, 0:HD], vp)

            # ---- attention + proj, blocked over queries ----
            SC = float(1.0 / np.sqrt(HD))
            for ib in range(NIB):
                ibs = ib * IB
                pv_ps = ps_pv.tile([HD + 1, IB], f32, tag="pv")
                for jc in range(NJC):
                    st_ps = ps_st.tile([P, IB], f32, tag="st")
                    # two concurrent row-group matmuls (partitions 0-63 and
                    # 64-127 hold identical k/q copies)
                    nc.tensor.matmul(
                        st_ps[:, 0:512],
                        lhsT=kk_sb[0:HD, jc * P:(jc + 1) * P],
                        rhs=qq_sb[0:HD, ibs: ibs + 512],
                        start=True, stop=True,
                    )
                    nc.tensor.matmul(
                        st_ps[:, 512:1024],
                        lhsT=kk_sb[HD:P, jc * P:(jc + 1) * P],
                        rhs=qq_sb[HD:P, ibs + 512: ibs + 1024],
                        start=True, stop=True,
                    )
                    pt = ptp.tile([P, IB], adt, tag="pt")
                    nc.scalar.activation(out=pt, in_=st_ps, func=AF.Exp, scale=SC)
                    for n2 in range(IB // 512):
                        nc.tensor.matmul(
                            pv_ps[:, n2 * 512:(n2 + 1) * 512],
                            lhsT=vt_sb[:, jc, :],
                            rhs=pt[:, n2 * 512:(n2 + 1) * 512],
                            start=(jc == 0), stop=(jc == NJC - 1),
                        )
                oh16 = ohp.tile([HD, IB], pdt, tag="oh16")
                nc.vector.tensor_copy(oh16, pv_ps[0:HD, :])
                den_sb = ohp.tile([1, IB], f32, tag="den")
                nc.vector.tensor_copy(den_sb, pv_ps[HD:HD + 1, :])
                nc.sync.dma_start(den_d[ib:ib + 1, :], den_sb)
                for mt in range(2):
                    for n2 in range(IB // 512):
                        pp = ps_sm.tile([P, 512], f32, tag="small", name="pp")
                        nc.tensor.matmul(
                            pp,
                            lhsT=wp_sb[:, mt * P:(mt + 1) * P],
                            rhs=oh16[:, n2 * 512:(n2 + 1) * 512],
                            start=True, stop=True,
                        )
                        sg = ostage.tile([P, 512], f32, tag="ostage")
                        nc.vector.tensor_copy(sg, pp)
                        nc.sync.dma_start(
                            out_d[:, mt, ibs + n2 * 512: ibs + (n2 + 1) * 512], sg
                        )
    nc.compile()
    return nc


def get_module(mm=MM_MODE):
    if mm not in _module_cache:
        _module_cache[mm] = _build_module(mm)
    return _module_cache[mm]


def _group_mats():
    gmat = np.zeros((P, 2, GROUPS), np.float32)
    gbc = np.zeros((GROUPS, 2, P), np.float32)
    for po in range(2):
        for pi in range(P):
            g = (po * P + pi) // 8
            gmat[pi, po, g] = 1.0 / 8.0
            gbc[g, po, pi] = 1.0
    return gmat, gbc


def make_in_maps(x, gn_weight, gn_bias, qkv_weight, qkv_bias,
                 proj_weight=None, mm=None):
    mm = mm or MM_MODE
    wp_np = np.float16 if mm == "f16" else np.float32
    x = np.asarray(x, np.float32)
    gn_weight = np.asarray(gn_weight, np.float32)
    gn_bias = np.asarray(gn_bias, np.float32)
    qkv_weight = np.asarray(qkv_weight, np.float32)
    qkv_bias = np.asarray(qkv_bias, np.float32)
    gmat, gbc = _group_mats()
    gnw = np.ascontiguousarray(gn_weight.reshape(2, P, 1).transpose(1, 0, 2))
    gnb = np.ascontiguousarray(gn_bias.reshape(2, P, 1).transpose(1, 0, 2))

    def wslice(row0):
        w = qkv_weight[row0:row0 + HD, :]            # [64, 256]
        return np.ascontiguousarray(
            w.T.reshape(2, P, HD).transpose(1, 0, 2).astype(wp_np))

    def bias2(off):
        b = qkv_bias[off:off + HD].reshape(HD, 1)
        return np.ascontiguousarray(np.vstack([b, b]).astype(np.float32))

    wps = [None] * NH
    if proj_weight is not None:
        pw = np.asarray(proj_weight, np.float32)
        wps = [np.ascontiguousarray(
            pw[:, h * HD:(h + 1) * HD].T.astype(wp_np)) for h in range(NH)]

    in_maps = []
    for b in range(B):
        xt = np.ascontiguousarray(x[b].reshape(2, P, HW).transpose(1, 0, 2))
        for h in range(NH):
            in_maps.append({
                "x": xt,
                "wq": wslice(h * HD),
                "wk": wslice(C + h * HD),
                "wv": wslice(2 * C + h * HD),
                "wp": wps[h],
                "bq": bias2(h * HD),
                "bk": bias2(C + h * HD),
                "gnw": gnw,
                "gnb": gnb,
                "gmat": gmat,
                "gbc": gbc,
            })
    return in_maps


def combine_outputs(results, x, qkv_bias, proj_weight, proj_bias):
    """results: list of 8 dicts with 'out' [128,2,HW] and 'den' [NIB,IB]."""
    x = np.asarray(x, np.float32)
    qkv_bias = np.asarray(qkv_bias, np.float32)
    proj_weight = np.asarray(proj_weight, np.float32)
    proj_bias = np.asarray(proj_bias, np.float32)
    y = np.empty((B, C, H, W), np.float32)
    for b in range(B):
        acc = x[b].reshape(C, HW) + proj_bias[:, None]
        for h in range(NH):
            r = results[b * NH + h]
            part = np.asarray(r["out"]).transpose(1, 0, 2).reshape(C, HW)
            den = np.asarray(r["den"]).reshape(HW)
            bv = qkv_bias[2 * C + h * HD: 2 * C + (h + 1) * HD]
            ch = proj_weight[:, h * HD:(h + 1) * HD] @ bv
            acc = acc + part / den[None, :] + ch[:, None]
        y[b] = acc.reshape(C, H, W)
    return y


def kernel(x, gn_weight, gn_bias, qkv_weight, qkv_bias, proj_weight, proj_bias):
    from concourse.bass_utils import run_bass_kernel_spmd

    nc = get_module()
    in_maps = make_in_maps(x, gn_weight, gn_bias, qkv_weight, qkv_bias,
                           proj_weight=proj_weight)
    res = run_bass_kernel_spmd(nc, in_maps, core_ids=list(range(NCORES)))
    return combine_outputs(res.results, x, qkv_bias, proj_weight, proj_bias)


# revision 21
# speedup vs baseline: 1.2042x; 1.2042x over previous
"""Trainium2 Bass kernel for nn_Attention2d.

Computation: GroupNorm(32 groups) -> 1x1 qkv conv -> 4-head attention over
H*W=4096 positions -> 1x1 proj conv -> residual add.

Sharding: one (batch, head) pair per NeuronCore (B=2 x NH=4 = 8 cores).
Each core:
  - GroupNorm of its batch slice (replicated across the 4 cores of a batch)
  - its head's q/k (with bias) and v^T (no bias)
  - S^T = k^T q in [keys-on-partitions, queries-on-free] layout
    (no max-subtraction: |S/8| <~ 6 so exp is safe in fp32)
  - P^T = exp(S^T/8); PV via matmul with lhsT = [v^T | ones]  -> the ones
    column yields the softmax denominators for free (row 64 of the output)
  - proj partial = Wp[:, head]^T @ PV_raw  (un-normalized)
Host: out[b] = x[b] + proj_bias + sum_h (partial_h / denom_h + Wp_h @ bv_h)
(per-query softmax normalization and the v-bias term commute through proj).

PE-array packing: the S matmuls contract over only hd=64 partitions, so q and
k are duplicated onto partitions 64..127 (by col-packed qkv matmuls that cost
no extra PE time) and each S^T tile is computed as two concurrent matmuls on
row-groups (0,0) and (64,0) -- 2x effective S throughput.

Matmul dtypes: qkv/attention/proj matmuls use float16 operands (1 cycle/row,
fast weight loads, 11-bit mantissa); GroupNorm matmuls and the softmax
denominators stay exact fp32.
"""

import numpy as np

B, C, H, W = 2, 256, 64, 64
HW = H * W           # 4096
GROUPS = 32
NH = 4
HD = C // NH         # 64
EPS = 1e-5
P = 128
IB = 1024            # query block (PSUM-sized)
NIB = HW // IB       # 4
NJC = HW // P        # 32 key chunks
NCORES = B * NH

# "f32": exact fp32 everywhere (slow). "f32r": float32r qkv+attention
# (TF32-like rounding, ~3 cycles/row on HW). "f16": float16 operands.
MM_MODE = "f16"

_module_cache = {}


def _build_module(mm=MM_MODE):
    import concourse.bacc as bacc
    import concourse.tile as tile
    import concourse.mybir as mybir

    dt = mybir.dt
    f32 = dt.float32
    AF = mybir.ActivationFunctionType
    OP = mybir.AluOpType
    if mm == "f32":
        wdt, adt = f32, f32
    elif mm == "f32r":
        wdt, adt = dt.float32r, dt.float32r
    elif mm == "f16":
        wdt, adt = dt.float16, dt.float16
    else:
        raise ValueError(mm)
    pdt = dt.float16 if mm == "f16" else f32

    nc = bacc.Bacc(trn_type="TRN2", target_bir_lowering=False, debug=False)

    # ---- DRAM I/O (per-core tensors; host prepares layouts) ----
    # channel layout everywhere: c = po*128 + pi  ->  [pi, po, ...]
    x_d = nc.dram_tensor("x", [P, 2, HW], f32, kind="ExternalInput").ap()
    # packed qkv weight slices: [wq | wk | wv] along the last axis
    wqkv_d = nc.dram_tensor("wqkv", [P, 2, 3 * HD], wdt, kind="ExternalInput").ap()
    wp_d = nc.dram_tensor("wp", [HD, C], pdt, kind="ExternalInput").ap()
    # packed per-partition aux columns:
    # 0: bq (x2 halves), 1: bk, 2+po: gnw, 4+po: gnb, 6+32*po: gmat
    NAUX = 6 + 2 * GROUPS
    aux_d = nc.dram_tensor("aux", [P, NAUX], f32, kind="ExternalInput").ap()
    gbc_d = nc.dram_tensor("gbc", [GROUPS, 2, P], f32, kind="ExternalInput").ap()
    out_d = nc.dram_tensor("out", [P, 2, HW], f32, kind="ExternalOutput").ap()
    den_d = nc.dram_tensor("den", [NIB, IB], f32, kind="ExternalOutput").ap()

    with tile.TileContext(nc) as tc:
        with (
            tc.tile_pool(name="const", bufs=1) as const,
            tc.tile_pool(name="big", bufs=1) as big,
            tc.tile_pool(name="tmp", bufs=3) as tmp,
            tc.tile_pool(name="pt", bufs=4) as ptp,
            tc.tile_pool(name="oh", bufs=2) as ohp,
            tc.tile_pool(name="ostage", bufs=3) as ostage,
            tc.tile_pool(name="ps_st", bufs=2, space="PSUM") as ps_st,
            tc.tile_pool(name="ps_pv", bufs=1, space="PSUM") as ps_pv,
            tc.tile_pool(name="ps_sm", bufs=2, space="PSUM") as ps_sm,
        ):
            eps_sb = const.tile([GROUPS, 1], f32)
            nc.vector.memset(eps_sb, EPS)
            ones_sb = const.tile([P, 1], f32)
            nc.vector.memset(ones_sb, 1.0)
            # Touch Exp immediately so walrus's single ACT table load runs
            # during the DMA-in phase (GroupNorm's rsqrt is DVE-only).
            warm_sb = tmp.tile([GROUPS, 1], f32, tag="warm")
            nc.scalar.activation(out=warm_sb, in_=eps_sb, func=AF.Exp, scale=1.0)

            # ---- x first (chunked so stats start early), then weights ----
            x_sb = big.tile([P, 2, HW], f32)
            stats = [tmp.tile([P, 8, 6], f32, tag=f"bnstats{po}",
                              name=f"stats{po}")
                     for po in range(2)]
            for c in range(2):
                cs = slice(c * (HW // 2), (c + 1) * (HW // 2))
                for po in range(2):
                    nc.sync.dma_start(x_sb[:, po, cs], x_d[:, po, cs])
                    for s in range(4 * c, 4 * c + 4):
                        nc.vector.bn_stats(
                            out=stats[po][:, s, :],
                            in_=x_sb[:, po, s * 512:(s + 1) * 512],
                        )
            wqkv_sb = const.tile([P, 2, 3 * HD], wdt)
            nc.sync.dma_start(wqkv_sb, wqkv_d)
            aux_sb = const.tile([P, NAUX], f32)
            nc.sync.dma_start(aux_sb, aux_d)
            gbc_sb = const.tile([GROUPS, 2, P], f32)
            nc.sync.dma_start(gbc_sb, gbc_d)
            wp_sb = const.tile([HD, C], pdt)
            nc.sync.dma_start(wp_sb, wp_d)
            wq_sb = wqkv_sb[:, :, 0:HD]
            wk_sb = wqkv_sb[:, :, HD:2 * HD]
            wv_sb = wqkv_sb[:, :, 2 * HD:3 * HD]
            bq_sb = aux_sb[:, 0:1]
            bk_sb = aux_sb[:, 1:2]

            mv = []
            for po in range(2):
                mvp = const.tile([P, 2], f32, tag=f"mv{po}")
                nc.vector.bn_aggr(out=mvp, in_=stats[po])
                msq = tmp.tile([P, 1], f32, tag="msq")
                nc.vector.tensor_mul(msq, mvp[:, 0:1], mvp[:, 0:1])
                nc.vector.tensor_add(mvp[:, 1:2], mvp[:, 1:2], msq)
                mv.append(mvp)

            # group-level [mean_g, E[x^2]_g] via indicator matmul (values 1/8)
            gst_ps = ps_sm.tile([GROUPS, 2], f32, tag="small")
            nc.tensor.matmul(gst_ps, lhsT=aux_sb[:, 6:6 + GROUPS], rhs=mv[0],
                             start=True, stop=False)
            nc.tensor.matmul(gst_ps, lhsT=aux_sb[:, 6 + GROUPS:6 + 2 * GROUPS],
                             rhs=mv[1], start=False, stop=True)
            gst = const.tile([GROUPS, 2], f32)
            nc.vector.tensor_copy(gst, gst_ps)

            # var_g = E[x^2]_g - mean_g^2 + eps; rs = rsqrt(var) via the
            # bit-trick seed + 3 Newton iterations, all on the DVE (keeps the
            # ScalarE table set pinned to Exp for the whole kernel)
            varg = tmp.tile([GROUPS, 1], f32, tag="varg")
            nc.vector.tensor_mul(varg, gst[:, 0:1], gst[:, 0:1])
            nc.vector.tensor_sub(varg, gst[:, 1:2], varg)
            nc.vector.tensor_scalar_add(varg, varg, float(EPS))
            vhalf = tmp.tile([GROUPS, 1], f32, tag="vhalf")
            nc.vector.tensor_scalar_mul(vhalf, varg, 0.5)
            st = const.tile([GROUPS, 2], f32)  # [rs_g, -mu_g*rs_g]
            y = st[:, 0:1]
            i32 = mybir.dt.int32
            # y0 = bitcast(0x5f3759df - (bitcast(v) >> 1))
            nc.vector.tensor_scalar(
                out=y.bitcast(i32), in0=varg.bitcast(i32),
                scalar1=1, scalar2=None,
                op0=OP.logical_shift_right,
            )
            nc.vector.tensor_scalar(
                out=y.bitcast(i32), in0=y.bitcast(i32),
                scalar1=-1, scalar2=0x5F3759DF,
                op0=OP.mult, op1=OP.add,
            )
            tnr = tmp.tile([GROUPS, 1], f32, tag="tnr")
            for _ in range(3):
                nc.vector.tensor_mul(tnr, y, y)
                nc.vector.tensor_mul(tnr, tnr, vhalf)
                nc.vector.tensor_scalar(
                    out=tnr, in0=tnr, scalar1=-1.0, scalar2=1.5,
                    op0=OP.mult, op1=OP.add,
                )
                nc.vector.tensor_mul(y, y, tnr)
            nc.vector.tensor_mul(st[:, 1:2], gst[:, 0:1], st[:, 0:1])
            nc.vector.tensor_scalar_mul(st[:, 1:2], st[:, 1:2], -1.0)

            # broadcast to channels, apply gn weight/bias; xn = s*x + t
            xn_sb = big.tile([P, 2, HW], wdt)
            sca = []
            for po in range(2):
                stc_ps = ps_sm.tile([P, 2], f32, tag="small")
                nc.tensor.matmul(stc_ps, lhsT=gbc_sb[:, po, :], rhs=st,
                                 start=True, stop=True)
                sc = const.tile([P, 2], f32, tag=f"sca{po}")
                nc.vector.tensor_mul(sc[:, 0:1], stc_ps[:, 0:1],
                                     aux_sb[:, 2 + po:3 + po])
                nc.vector.tensor_mul(sc[:, 1:2], stc_ps[:, 1:2],
                                     aux_sb[:, 2 + po:3 + po])
                nc.vector.tensor_add(sc[:, 1:2], sc[:, 1:2],
                                     aux_sb[:, 4 + po:5 + po])
                sca.append(sc)
            for c in range(4):
                cs = slice(c * IB, (c + 1) * IB)
                for po in range(2):
                    nc.vector.tensor_scalar(
                        out=xn_sb[:, po, cs], in0=x_sb[:, po, cs],
                        scalar1=sca[po][:, 0:1], scalar2=sca[po][:, 1:2],
                        op0=OP.mult, op1=OP.add,
                    )

            # ---- qkv; q and k duplicated onto partitions 64..127 via
            # col-packed matmuls (concurrent on the PE array) ----
            qq_sb = big.tile([P, HW], adt)
            kk_sb = big.tile([P, HW], adt)
            vt_sb = big.tile([P, NJC, HD + 1], adt)
            nc.vector.tensor_copy(
                vt_sb[:, :, HD:HD + 1],
                ones_sb[:, None, :].to_broadcast([P, NJC, 1]),
            )
            for n in range(HW // 512):
                ns = slice(n * 512, (n + 1) * 512)
                for (wsb, bsb, dst) in ((wq_sb, bq_sb, qq_sb),
                                        (wk_sb, bk_sb, kk_sb)):
                    qp = ps_sm.tile([P, 512], f32, tag="small", name="qp")
                    for half in range(2):
                        hs = slice(half * HD, (half + 1) * HD)
                        nc.tensor.matmul(qp[hs, :], lhsT=wsb[:, 0, :],
                                         rhs=xn_sb[:, 0, ns],
                                         start=True, stop=False)
                        nc.tensor.matmul(qp[hs, :], lhsT=wsb[:, 1, :],
                                         rhs=xn_sb[:, 1, ns],
                                         start=False, stop=True)
                    nc.vector.tensor_scalar_add(dst[:, ns], qp, bsb)
            # v^T directly: [positions, head_dim], chunked by 128 positions
            for jc in range(NJC):
                js = slice(jc * P, (jc + 1) * P)
                vp = ps_sm.tile([P, HD], f32, tag="small", name="vp")
                nc.tensor.matmul(vp, lhsT=xn_sb[:, 0, js],
                                 rhs=wv_sb[:, 0, :], start=True, stop=False)
                nc.tensor.matmul(vp, lhsT=xn_sb[:, 1, js],
                                 rhs=wv_sb[:, 1, :], start=False, stop=True)
                nc.vector.tensor_copy(vt_sb[:, jc, 0:HD], vp)

            # ---- attention + proj, blocked over queries ----
            SC = float(1.0 / np.sqrt(HD))
            for ib in range(NIB):
                ibs = ib * IB
                pv_ps = ps_pv.tile([HD + 1, IB], f32, tag="pv")
                for jc in range(NJC):
                    st_ps = ps_st.tile([P, IB], f32, tag="st")
                    # two concurrent row-group matmuls (partitions 0-63 and
                    # 64-127 hold identical k/q copies)
                    nc.tensor.matmul(
                        st_ps[:, 0:512],
                        lhsT=kk_sb[0:HD, jc * P:(jc + 1) * P],
                        rhs=qq_sb[0:HD, ibs: ibs + 512],
                        start=True, stop=True,
                    )
                    nc.tensor.matmul(
                        st_ps[:, 512:1024],
                        lhsT=kk_sb[HD:P, jc * P:(jc + 1) * P],
                        rhs=qq_sb[HD:P, ibs + 512: ibs + 1024],
                        start=True, stop=True,
                    )
                    pt = ptp.tile([P, IB], adt, tag="pt")
                    nc.scalar.activation(out=pt, in_=st_ps, func=AF.Exp, scale=SC)
                    for n2 in range(IB // 512):
                        nc.tensor.matmul(
                            pv_ps[:, n2 * 512:(n2 + 1) * 512],
                            lhsT=vt_sb[:, jc, :],
                            rhs=pt[:, n2 * 512:(n2 + 1) * 512],
                            start=(jc == 0), stop=(jc == NJC - 1),
                        )
                oh16 = ohp.tile([HD, IB], pdt, tag="oh16")
                nc.vector.tensor_copy(oh16, pv_ps[0:HD, :])
                den_sb = ohp.tile([1, IB], f32, tag="den")
                nc.vector.tensor_copy(den_sb, pv_ps[HD:HD + 1, :])
                nc.sync.dma_start(den_d[ib:ib + 1, :], den_sb)
                for mt in range(2):
                    for n2 in range(IB // 512):
                        pp = ps_sm.tile([P, 512], f32, tag="small", name="pp")
                        nc.tensor.matmul(
                            pp,
                            lhsT=wp_sb[:, mt * P:(mt + 1) * P],
                            rhs=oh16[:, n2 * 512:(n2 + 1) * 512],
                            start=True, stop=True,
                        )
                        sg = ostage.tile([P, 512], f32, tag="ostage")
                        nc.vector.tensor_copy(sg, pp)
                        nc.sync.dma_start(
                            out_d[:, mt, ibs + n2 * 512: ibs + (n2 + 1) * 512], sg
                        )
    nc.compile()
    return nc


def get_module(mm=MM_MODE):
    if mm not in _module_cache:
        _module_cache[mm] = _build_module(mm)
    return _module_cache[mm]


def _group_mats():
    gmat = np.zeros((P, 2, GROUPS), np.float32)
    gbc = np.zeros((GROUPS, 2, P), np.float32)
    for po in range(2):
        for pi in range(P):
            g = (po * P + pi) // 8
            gmat[pi, po, g] = 1.0 / 8.0
            gbc[g, po, pi] = 1.0
    return gmat, gbc


def make_in_maps(x, gn_weight, gn_bias, qkv_weight, qkv_bias,
                 proj_weight=None, mm=None):
    mm = mm or MM_MODE
    wp_np = np.float16 if mm == "f16" else np.float32
    x = np.asarray(x, np.float32)
    gn_weight = np.asarray(gn_weight, np.float32)
    gn_bias = np.asarray(gn_bias, np.float32)
    qkv_weight = np.asarray(qkv_weight, np.float32)
    qkv_bias = np.asarray(qkv_bias, np.float32)
    gmat, gbc = _group_mats()
    gnw = np.ascontiguousarray(gn_weight.reshape(2, P, 1).transpose(1, 0, 2))
    gnb = np.ascontiguousarray(gn_bias.reshape(2, P, 1).transpose(1, 0, 2))

    def wslice(row0):
        w = qkv_weight[row0:row0 + HD, :]            # [64, 256]
        return w.T.reshape(2, P, HD).transpose(1, 0, 2)

    def bias2(off):
        b = qkv_bias[off:off + HD].reshape(HD, 1)
        return np.vstack([b, b])

    wps = [None] * NH
    if proj_weight is not None:
        pw = np.asarray(proj_weight, np.float32)
        wps = [np.ascontiguousarray(
            pw[:, h * HD:(h + 1) * HD].T.astype(wp_np)) for h in range(NH)]

    in_maps = []
    for b in range(B):
        xt = np.ascontiguousarray(x[b].reshape(2, P, HW).transpose(1, 0, 2))
        for h in range(NH):
            wqkv = np.concatenate(
                [wslice(h * HD), wslice(C + h * HD), wslice(2 * C + h * HD)],
                axis=2).astype(wp_np)
            aux = np.concatenate(
                [bias2(h * HD), bias2(C + h * HD),
                 gnw[:, 0, :], gnw[:, 1, :], gnb[:, 0, :], gnb[:, 1, :],
                 gmat[:, 0, :], gmat[:, 1, :]], axis=1).astype(np.float32)
            in_maps.append({
                "x": xt,
                "wqkv": np.ascontiguousarray(wqkv),
                "wp": wps[h],
                "aux": np.ascontiguousarray(aux),
                "gbc": gbc,
            })
    return in_maps


def combine_outputs(results, x, qkv_bias, proj_weight, proj_bias):
    """results: list of 8 dicts with 'out' [128,2,HW] and 'den' [NIB,IB]."""
    x = np.asarray(x, np.float32)
    qkv_bias = np.asarray(qkv_bias, np.float32)
    proj_weight = np.asarray(proj_weight, np.float32)
    proj_bias = np.asarray(proj_bias, np.float32)
    y = np.empty((B, C, H, W), np.float32)
    for b in range(B):
        acc = x[b].reshape(C, HW) + proj_bias[:, None]
        for h in range(NH):
            r = results[b * NH + h]
            part = np.asarray(r["out"]).transpose(1, 0, 2).reshape(C, HW)
            den = np.asarray(r["den"]).reshape(HW)
            bv = qkv_bias[2 * C + h * HD: 2 * C + (h + 1) * HD]
            ch = proj_weight[:, h * HD:(h + 1) * HD] @ bv
            acc = acc + part / den[None, :] + ch[:, None]
        y[b] = acc.reshape(C, H, W)
    return y


def kernel(x, gn_weight, gn_bias, qkv_weight, qkv_bias, proj_weight, proj_bias):
    from concourse.bass_utils import run_bass_kernel_spmd

    nc = get_module()
    in_maps = make_in_maps(x, gn_weight, gn_bias, qkv_weight, qkv_bias,
                           proj_weight=proj_weight)
    res = run_bass_kernel_spmd(nc, in_maps, core_ids=list(range(NCORES)))
    return combine_outputs(res.results, x, qkv_bias, proj_weight, proj_bias)


# revision 23
# speedup vs baseline: 1.2164x; 1.0101x over previous
"""Trainium2 Bass kernel for nn_Attention2d.

Computation: GroupNorm(32 groups) -> 1x1 qkv conv -> 4-head attention over
H*W=4096 positions -> 1x1 proj conv -> residual add.

Sharding: one (batch, head) pair per NeuronCore (B=2 x NH=4 = 8 cores).
Each core:
  - GroupNorm stats of its batch slice; the affine normalization is folded
    into the qkv weights (W' = W*s per input channel) and effective biases
    (b' = W@t + b), so the x-sized tensor is only cast to fp16 once
  - its head's q/k (with effective bias) and v^T (bias exported to host)
  - S^T = k^T q in [keys-on-partitions, queries-on-free] layout
    (no max-subtraction: |S/8| <~ 6 so exp is safe in fp32)
  - P^T = exp(S^T/8); PV via matmul with lhsT = [v^T | ones]  -> the ones
    column yields the softmax denominators for free (row 64 of the output)
  - proj partial = Wp[:, head]^T @ PV_raw  (un-normalized)
Host: out[b] = x[b] + proj_bias + sum_h (partial_h/denom_h + Wp_h @ bve_h)
(the softmax normalization and the constant v-bias commute through proj).

PE-array packing: the S matmuls contract over only hd=64 partitions, so q and
k are duplicated onto partitions 64..127 (by col-packed qkv matmuls that cost
no extra PE time) and each S^T tile is computed as two concurrent matmuls on
row-groups (0,0) and (64,0).

Matmul dtypes: qkv/attention/proj matmuls use float16 operands (1 cycle/row,
fast weight loads); GroupNorm matmuls, softmax denominators and all
reductions stay fp32. GroupNorm's rsqrt runs on the DVE (bit-trick seed +
Newton) so the ScalarE keeps a single Exp table set for the whole kernel.
"""

import numpy as np

B, C, H, W = 2, 256, 64, 64
HW = H * W           # 4096
GROUPS = 32
NH = 4
HD = C // NH         # 64
EPS = 1e-5
P = 128
IB = 1024            # query block (PSUM-sized)
NIB = HW // IB       # 4
NJC = HW // P        # 32 key chunks
NCORES = B * NH

# "f32": exact fp32 everywhere (slow). "f32r": float32r operands
# (TF32-like rounding, ~3 cycles/row on HW). "f16": float16 operands.
MM_MODE = "f16"

# aux column layout: 0 bq2, 1 bk2, 2 bv (rows 0:64), 3+po gnw, 5+po gnb,
# 7+32*po gmat
NAUX = 7 + 2 * GROUPS

_module_cache = {}


def _build_module(mm=MM_MODE):
    import concourse.bacc as bacc
    import concourse.tile as tile
    import concourse.mybir as mybir

    dt = mybir.dt
    f32 = dt.float32
    AF = mybir.ActivationFunctionType
    OP = mybir.AluOpType
    if mm == "f32":
        adt = f32
    elif mm == "f32r":
        adt = dt.float32r
    elif mm == "f16":
        adt = dt.float16
    else:
        raise ValueError(mm)

    nc = bacc.Bacc(trn_type="TRN2", target_bir_lowering=False, debug=False)

    # ---- DRAM I/O (per-core tensors; host prepares layouts) ----
    # channel layout everywhere: c = po*128 + pi  ->  [pi, po, ...]
    x_d = nc.dram_tensor("x", [P, 2, HW], f32, kind="ExternalInput").ap()
    # packed raw qkv weight slices: [wq | wk | wv] along the last axis, fp32
    wqkv_d = nc.dram_tensor("wqkv", [P, 2, 3 * HD], f32, kind="ExternalInput").ap()
    wp_d = nc.dram_tensor("wp", [HD, C], adt if mm == "f16" else f32,
                          kind="ExternalInput").ap()
    aux_d = nc.dram_tensor("aux", [P, NAUX], f32, kind="ExternalInput").ap()
    gbc_d = nc.dram_tensor("gbc", [GROUPS, 2, P], f32, kind="ExternalInput").ap()
    out_d = nc.dram_tensor("out", [P, 2, HW], f32, kind="ExternalOutput").ap()
    den_d = nc.dram_tensor("den", [NIB, IB], f32, kind="ExternalOutput").ap()
    bve_d = nc.dram_tensor("bve", [HD, 1], f32, kind="ExternalOutput").ap()

    with tile.TileContext(nc) as tc:
        with (
            tc.tile_pool(name="const", bufs=1) as const,
            tc.tile_pool(name="big", bufs=1) as big,
            tc.tile_pool(name="tmp", bufs=3) as tmp,
            tc.tile_pool(name="pt", bufs=4) as ptp,
            tc.tile_pool(name="oh", bufs=2) as ohp,
            tc.tile_pool(name="ostage", bufs=3) as ostage,
            tc.tile_pool(name="ps_st", bufs=2, space="PSUM") as ps_st,
            tc.tile_pool(name="ps_pv", bufs=1, space="PSUM") as ps_pv,
            tc.tile_pool(name="ps_sm", bufs=2, space="PSUM") as ps_sm,
        ):
            eps_sb = const.tile([GROUPS, 1], f32)
            nc.vector.memset(eps_sb, EPS)
            ones_sb = const.tile([P, 1], f32)
            nc.vector.memset(ones_sb, 1.0)
            # Touch Exp immediately so walrus's single ACT table load runs
            # during the DMA-in phase.
            warm_sb = tmp.tile([GROUPS, 1], f32, tag="warm")
            nc.scalar.activation(out=warm_sb, in_=eps_sb, func=AF.Exp, scale=1.0)

            # ---- x first (chunked: stats + fp16 cast start early) ----
            x_sb = big.tile([P, 2, HW], f32)
            x16 = big.tile([P, 2, HW], adt)
            stats = [tmp.tile([P, 8, 6], f32, tag=f"bnstats{po}",
                              name=f"stats{po}")
                     for po in range(2)]
            for c in range(2):
                cs = slice(c * (HW // 2), (c + 1) * (HW // 2))
                for po in range(2):
                    nc.sync.dma_start(x_sb[:, po, cs], x_d[:, po, cs])
                    for s in range(4 * c, 4 * c + 4):
                        nc.vector.bn_stats(
                            out=stats[po][:, s, :],
                            in_=x_sb[:, po, s * 512:(s + 1) * 512],
                        )
                    nc.scalar.copy(x16[:, po, cs], x_sb[:, po, cs])
            wqkv_sb = const.tile([P, 2, 3 * HD], f32)
            nc.sync.dma_start(wqkv_sb, wqkv_d)
            aux_sb = const.tile([P, NAUX], f32)
            nc.sync.dma_start(aux_sb, aux_d)
            gbc_sb = const.tile([GROUPS, 2, P], f32)
            nc.sync.dma_start(gbc_sb, gbc_d)
            wp_sb = const.tile([HD, C], adt if mm == "f16" else f32)
            nc.sync.dma_start(wp_sb, wp_d)

            mv = []
            for po in range(2):
                mvp = const.tile([P, 2], f32, tag=f"mv{po}")
                nc.vector.bn_aggr(out=mvp, in_=stats[po])
                msq = tmp.tile([P, 1], f32, tag="msq")
                nc.vector.tensor_mul(msq, mvp[:, 0:1], mvp[:, 0:1])
                nc.vector.tensor_add(mvp[:, 1:2], mvp[:, 1:2], msq)
                mv.append(mvp)

            # group-level [mean_g, E[x^2]_g] via indicator matmul (values 1/8)
            gst_ps = ps_sm.tile([GROUPS, 2], f32, tag="small")
            nc.tensor.matmul(gst_ps, lhsT=aux_sb[:, 7:7 + GROUPS], rhs=mv[0],
                             start=True, stop=False)
            nc.tensor.matmul(gst_ps, lhsT=aux_sb[:, 7 + GROUPS:7 + 2 * GROUPS],
                             rhs=mv[1], start=False, stop=True)
            gst = const.tile([GROUPS, 2], f32)
            nc.vector.tensor_copy(gst, gst_ps)

            # var_g = E[x^2]_g - mean_g^2 + eps; rs = rsqrt(var) via the
            # bit-trick seed + 3 Newton iterations, all on the DVE
            varg = tmp.tile([GROUPS, 1], f32, tag="varg")
            nc.vector.tensor_mul(varg, gst[:, 0:1], gst[:, 0:1])
            nc.vector.tensor_sub(varg, gst[:, 1:2], varg)
            nc.vector.tensor_scalar_add(varg, varg, float(EPS))
            vhalf = tmp.tile([GROUPS, 1], f32, tag="vhalf")
            nc.vector.tensor_scalar_mul(vhalf, varg, 0.5)
            st = const.tile([GROUPS, 2], f32)  # [rs_g, -mu_g*rs_g]
            y = st[:, 0:1]
            i32 = mybir.dt.int32
            nc.vector.tensor_scalar(
                out=y.bitcast(i32), in0=varg.bitcast(i32),
                scalar1=1, scalar2=None, op0=OP.logical_shift_right,
            )
            nc.vector.tensor_scalar(
                out=y.bitcast(i32), in0=y.bitcast(i32),
                scalar1=-1, scalar2=0x5F3759DF, op0=OP.mult, op1=OP.add,
            )
            tnr = tmp.tile([GROUPS, 1], f32, tag="tnr")
            for _ in range(3):
                nc.vector.tensor_mul(tnr, y, y)
                nc.vector.tensor_mul(tnr, tnr, vhalf)
                nc.vector.tensor_scalar(
                    out=tnr, in0=tnr, scalar1=-1.0, scalar2=1.5,
                    op0=OP.mult, op1=OP.add,
                )
                nc.vector.tensor_mul(y, y, tnr)
            nc.vector.tensor_mul(st[:, 1:2], gst[:, 0:1], st[:, 0:1])
            nc.vector.tensor_scalar_mul(st[:, 1:2], st[:, 1:2], -1.0)

            # per-channel [s_c, t_c] (gn weight/bias applied)
            sca = []
            for po in range(2):
                stc_ps = ps_sm.tile([P, 2], f32, tag="small")
                nc.tensor.matmul(stc_ps, lhsT=gbc_sb[:, po, :], rhs=st,
                                 start=True, stop=True)
                sc = const.tile([P, 2], f32, tag=f"sca{po}")
                nc.vector.tensor_mul(sc[:, 0:1], stc_ps[:, 0:1],
                                     aux_sb[:, 3 + po:4 + po])
                nc.vector.tensor_mul(sc[:, 1:2], stc_ps[:, 1:2],
                                     aux_sb[:, 3 + po:4 + po])
                nc.vector.tensor_add(sc[:, 1:2], sc[:, 1:2],
                                     aux_sb[:, 5 + po:6 + po])
                sca.append(sc)

            # scaled fp16 qkv weights: W'[.,c] = W[.,c] * s_c
            wsc = const.tile([P, 2, 3 * HD], adt)
            for po in range(2):
                nc.vector.tensor_scalar_mul(wsc[:, po, :], wqkv_sb[:, po, :],
                                            sca[po][:, 0:1])

            # effective biases: b' = W @ t + b_raw.
            # q and k adj land on both partition halves via col-packed N=1
            # matmuls; v adj on rows 0:64 only.
            bqe = const.tile([P, 1], f32)
            bke = const.tile([P, 1], f32)
            bve = const.tile([P, 1], f32)
            for (wofs, dst, rawcol) in ((0, bqe, 0), (HD, bke, 1),
                                        (2 * HD, bve, 2)):
                bp = ps_sm.tile([P, 1], f32, tag="small", name="bp")
                halves = (0, 1) if rawcol < 2 else (0,)
                for half in halves:
                    hs = slice(half * HD, (half + 1) * HD)
                    nc.tensor.matmul(bp[hs, :],
                                     lhsT=wqkv_sb[:, 0, wofs:wofs + HD],
                                     rhs=sca[0][:, 1:2], start=True, stop=False)
                    nc.tensor.matmul(bp[hs, :],
                                     lhsT=wqkv_sb[:, 1, wofs:wofs + HD],
                                     rhs=sca[1][:, 1:2], start=False, stop=True)
                nc.vector.tensor_add(dst[:HD * len(halves), :],
                                     bp[:HD * len(halves), :],
                                     aux_sb[:HD * len(halves),
                                            rawcol:rawcol + 1])
            nc.sync.dma_start(bve_d, bve[0:HD, :])

            # ---- qkv on x16; q and k duplicated onto partitions 64..127
            # via col-packed matmuls (concurrent on the PE array) ----
            qq_sb = big.tile([P, HW], adt)
            kk_sb = big.tile([P, HW], adt)
            vt_sb = big.tile([P, NJC, HD + 1], adt)
            nc.vector.tensor_copy(
                vt_sb[:, :, HD:HD + 1],
                ones_sb[:, None, :].to_broadcast([P, NJC, 1]),
            )
            for n in range(HW // 512):
                ns = slice(n * 512, (n + 1) * 512)
                for (wofs, bsb, dst) in ((0, bqe, qq_sb), (HD, bke, kk_sb)):
                    qp = ps_sm.tile([P, 512], f32, tag="small", name="qp")
                    for half in range(2):
                        hs = slice(half * HD, (half + 1) * HD)
                        nc.tensor.matmul(qp[hs, :],
                                         lhsT=wsc[:, 0, wofs:wofs + HD],
                                         rhs=x16[:, 0, ns],
                                         start=True, stop=False)
                        nc.tensor.matmul(qp[hs, :],
                                         lhsT=wsc[:, 1, wofs:wofs + HD],
                                         rhs=x16[:, 1, ns],
                                         start=False, stop=True)
                    nc.vector.tensor_scalar_add(dst[:, ns], qp, bsb)
            # v^T directly: [positions, head_dim], chunked by 128 positions
            for jc in range(NJC):
                js = slice(jc * P, (jc + 1) * P)
                vp = ps_sm.tile([P, HD], f32, tag="small", name="vp")
                nc.tensor.matmul(vp, lhsT=x16[:, 0, js],
                                 rhs=wsc[:, 0, 2 * HD:3 * HD],
                                 start=True, stop=False)
                nc.tensor.matmul(vp, lhsT=x16[:, 1, js],
                                 rhs=wsc[:, 1, 2 * HD:3 * HD],
                                 start=False, stop=True)
                nc.vector.tensor_copy(vt_sb[:, jc, 0:HD], vp)

            # ---- attention + proj, blocked over queries. The previous
            # block's epilogue is emitted after the first two exp's of the
            # next block so the ScalarE never stalls at block boundaries ----
            SC = float(1.0 / np.sqrt(HD))
            pend = []

            def emit_epilogue():
                if not pend:
                    return
                ib0, pv0 = pend.pop()
                ibs0 = ib0 * IB
                oh16 = ohp.tile([HD, IB], adt, tag="oh16", name="oh16")
                nc.vector.tensor_copy(oh16, pv0[0:HD, :])
                den_sb = ohp.tile([1, IB], f32, tag="den", name="den_sb")
                nc.vector.tensor_copy(den_sb, pv0[HD:HD + 1, :])
                nc.sync.dma_start(den_d[ib0:ib0 + 1, :], den_sb)
                for mt in range(2):
                    for n2 in range(IB // 512):
                        pp = ps_sm.tile([P, 512], f32, tag="small", name="pp")
                        nc.tensor.matmul(
                            pp,
                            lhsT=wp_sb[:, mt * P:(mt + 1) * P],
                            rhs=oh16[:, n2 * 512:(n2 + 1) * 512],
                            start=True, stop=True,
                        )
                        sg = ostage.tile([P, 512], f32, tag="ostage", name="sg")
                        nc.vector.tensor_copy(sg, pp)
                        nc.sync.dma_start(
                            out_d[:, mt, ibs0 + n2 * 512: ibs0 + (n2 + 1) * 512],
                            sg)

            for ib in range(NIB):
                ibs = ib * IB
                pts = {}
                # S + exp for the first two key-chunks before the previous
                # block's epilogue claims the PE
                for jc in range(2):
                    st_ps = ps_st.tile([P, IB], f32, tag="st", name="st_ps")
                    nc.tensor.matmul(
                        st_ps[:, 0:512],
                        lhsT=kk_sb[0:HD, jc * P:(jc + 1) * P],
                        rhs=qq_sb[0:HD, ibs: ibs + 512],
                        start=True, stop=True,
                    )
                    nc.tensor.matmul(
                        st_ps[:, 512:1024],
                        lhsT=kk_sb[HD:P, jc * P:(jc + 1) * P],
                        rhs=qq_sb[HD:P, ibs + 512: ibs + 1024],
                        start=True, stop=True,
                    )
                    pt = ptp.tile([P, IB], adt, tag="pt", name="pt")
                    nc.scalar.activation(out=pt, in_=st_ps, func=AF.Exp,
                                         scale=SC)
                    pts[jc] = pt
                emit_epilogue()
                pv_ps = ps_pv.tile([HD + 1, IB], f32, tag="pv", name="pv_ps")
                for jc in range(NJC):
                    if jc in pts:
                        pt = pts.pop(jc)
                    else:
                        st_ps = ps_st.tile([P, IB], f32, tag="st", name="st_ps")
                        nc.tensor.matmul(
                            st_ps[:, 0:512],
                            lhsT=kk_sb[0:HD, jc * P:(jc + 1) * P],
                            rhs=qq_sb[0:HD, ibs: ibs + 512],
                            start=True, stop=True,
                        )
                        nc.tensor.matmul(
                            st_ps[:, 512:1024],
                            lhsT=kk_sb[HD:P, jc * P:(jc + 1) * P],
                            rhs=qq_sb[HD:P, ibs + 512: ibs + 1024],
                            start=True, stop=True,
                        )
                        pt = ptp.tile([P, IB], adt, tag="pt", name="pt")
                        nc.scalar.activation(out=pt, in_=st_ps, func=AF.Exp,
                                             scale=SC)
                    for n2 in range(IB // 512):
                        nc.tensor.matmul(
                            pv_ps[:, n2 * 512:(n2 + 1) * 512],
                            lhsT=vt_sb[:, jc, :],
                            rhs=pt[:, n2 * 512:(n2 + 1) * 512],
                            start=(jc == 0), stop=(jc == NJC - 1),
                        )
                pend.append((ib, pv_ps))
            emit_epilogue()
    nc.compile()
    return nc


def get_module(mm=MM_MODE):
    if mm not in _module_cache:
        _module_cache[mm] = _build_module(mm)
    return _module_cache[mm]


def _group_mats():
    gmat = np.zeros((P, 2, GROUPS), np.float32)
    gbc = np.zeros((GROUPS, 2, P), np.float32)
    for po in range(2):
        for pi in range(P):
            g = (po * P + pi) // 8
            gmat[pi, po, g] = 1.0 / 8.0
            gbc[g, po, pi] = 1.0
    return gmat, gbc


def make_in_maps(x, gn_weight, gn_bias, qkv_weight, qkv_bias,
                 proj_weight=None, mm=None):
    mm = mm or MM_MODE
    wp_np = np.float16 if mm == "f16" else np.float32
    x = np.asarray(x, np.float32)
    gn_weight = np.asarray(gn_weight, np.float32)
    gn_bias = np.asarray(gn_bias, np.float32)
    qkv_weight = np.asarray(qkv_weight, np.float32)
    qkv_bias = np.asarray(qkv_bias, np.float32)
    gmat, gbc = _group_mats()
    gnw = np.ascontiguousarray(gn_weight.reshape(2, P).T)   # [128, 2]
    gnb = np.ascontiguousarray(gn_bias.reshape(2, P).T)

    def wslice(row0):
        w = qkv_weight[row0:row0 + HD, :]            # [64, 256]
        return w.T.reshape(2, P, HD).transpose(1, 0, 2)

    def bias2(off):
        b = qkv_bias[off:off + HD].reshape(HD, 1)
        return np.vstack([b, b])

    wps = [None] * NH
    if proj_weight is not None:
        pw = np.asarray(proj_weight, np.float32)
        wps = [np.ascontiguousarray(
            pw[:, h * HD:(h + 1) * HD].T.astype(wp_np)) for h in range(NH)]

    in_maps = []
    for b in range(B):
        xt = np.ascontiguousarray(x[b].reshape(2, P, HW).transpose(1, 0, 2))
        for h in range(NH):
            wqkv = np.concatenate(
                [wslice(h * HD), wslice(C + h * HD), wslice(2 * C + h * HD)],
                axis=2).astype(np.float32)
            bv = np.zeros((P, 1), np.float32)
            bv[0:HD, 0] = qkv_bias[2 * C + h * HD: 2 * C + (h + 1) * HD]
            aux = np.concatenate(
                [bias2(h * HD), bias2(C + h * HD), bv,
                 gnw[:, 0:1], gnw[:, 1:2], gnb[:, 0:1], gnb[:, 1:2],
                 gmat[:, 0, :], gmat[:, 1, :]], axis=1).astype(np.float32)
            in_maps.append({
                "x": xt,
                "wqkv": np.ascontiguousarray(wqkv),
                "wp": wps[h],
                "aux": np.ascontiguousarray(aux),
                "gbc": gbc,
            })
    return in_maps


def combine_outputs(results, x, proj_weight, proj_bias):
    """results: 8 dicts with 'out' [128,2,HW], 'den' [NIB,IB], 'bve' [HD,1]."""
    x = np.asarray(x, np.float32)
    proj_weight = np.asarray(proj_weight, np.float32)
    proj_bias = np.asarray(proj_bias, np.float32)
    y = np.empty((B, C, H, W), np.float32)
    for b in range(B):
        acc = x[b].reshape(C, HW) + proj_bias[:, None]
        for h in range(NH):
            r = results[b * NH + h]
            part = np.asarray(r["out"]).transpose(1, 0, 2).reshape(C, HW)
            den = np.asarray(r["den"]).reshape(HW)
            bve = np.asarray(r["bve"]).reshape(HD)
            ch = proj_weight[:, h * HD:(h + 1) * HD] @ bve
            acc = acc + part / den[None, :] + ch[:, None]
        y[b] = acc.reshape(C, H, W)
    return y


def kernel(x, gn_weight, gn_bias, qkv_weight, qkv_bias, proj_weight, proj_bias):
    from concourse.bass_utils import run_bass_kernel_spmd

    nc = get_module()
    in_maps = make_in_maps(x, gn_weight, gn_bias, qkv_weight, qkv_bias,
                           proj_weight=proj_weight)
    res = run_bass_kernel_spmd(nc, in_maps, core_ids=list(range(NCORES)))
    return combine_outputs(res.results, x, proj_weight, proj_bias)


# revision 27
# speedup vs baseline: 1.2334x; 1.0139x over previous
"""Trainium2 Bass kernel for nn_Attention2d.

Computation: GroupNorm(32 groups) -> 1x1 qkv conv -> 4-head attention over
H*W=4096 positions -> 1x1 proj conv -> residual add.

Sharding: one (batch, head) pair per NeuronCore (B=2 x NH=4 = 8 cores).
Each core:
  - GroupNorm stats of its batch slice; the affine normalization is folded
    into the qkv weights (W' = W*s per input channel) and effective biases
    (b' = W@t + b), so the x-sized tensor is only cast to fp16 once
  - its head's q/k (with effective bias) and v^T (bias exported to host)
  - S^T = k^T q in [keys-on-partitions, queries-on-free] layout
    (no max-subtraction: |S/8| <~ 6 so exp is safe in fp32)
  - P^T = exp(S^T/8); PV via matmul with lhsT = [v^T | ones]  -> the ones
    column yields the softmax denominators for free (row 64 of the output)
  - proj partial = Wp[:, head]^T @ PV_raw  (un-normalized)
Host: out[b] = x[b] + proj_bias + sum_h (partial_h/denom_h + Wp_h @ bve_h)
(the softmax normalization and the constant v-bias commute through proj).

PE-array packing: the S matmuls contract over only hd=64 partitions, so q and
k are duplicated onto partitions 64..127 (by col-packed qkv matmuls that cost
no extra PE time) and each S^T tile is computed as two concurrent matmuls on
row-groups (0,0) and (64,0).

Matmul dtypes: qkv/attention/proj matmuls use float16 operands (1 cycle/row,
fast weight loads); GroupNorm matmuls, softmax denominators and all
reductions stay fp32. GroupNorm's rsqrt runs on the DVE (bit-trick seed +
Newton) so the ScalarE keeps a single Exp table set for the whole kernel.
"""

import numpy as np

B, C, H, W = 2, 256, 64, 64
HW = H * W           # 4096
GROUPS = 32
NH = 4
HD = C // NH         # 64
EPS = 1e-5
P = 128
IB = 1024            # query block (PSUM-sized)
NIB = HW // IB       # 4
NJC = HW // P        # 32 key chunks
NCORES = B * NH

# "f32": exact fp32 everywhere (slow). "f32r": float32r operands
# (TF32-like rounding, ~3 cycles/row on HW). "f16": float16 operands.
MM_MODE = "f16"

# aux column layout: 0 bq2, 1 bk2, 2 bv (rows 0:64), 3+po gnw, 5+po gnb,
# 7+32*po gmat
NAUX = 7 + 2 * GROUPS

_module_cache = {}


def _build_module(mm=MM_MODE):
    import concourse.bacc as bacc
    import concourse.tile as tile
    import concourse.mybir as mybir

    dt = mybir.dt
    f32 = dt.float32
    AF = mybir.ActivationFunctionType
    OP = mybir.AluOpType
    if mm == "f32":
        adt = f32
    elif mm == "f32r":
        adt = dt.float32r
    elif mm == "f16":
        adt = dt.float16
    else:
        raise ValueError(mm)

    nc = bacc.Bacc(trn_type="TRN2", target_bir_lowering=False, debug=False)

    # ---- DRAM I/O (per-core tensors; host prepares layouts) ----
    # channel layout everywhere: c = po*128 + pi  ->  [pi, po, ...]
    # x arrives already cast to the attention dtype (host-side cast); the
    # GroupNorm statistics absorb the rounding (it averages out over 32k
    # elements per group).
    x_d = nc.dram_tensor("x", [P, 2, HW], adt, kind="ExternalInput").ap()
    # packed raw qkv weight slices: [wq | wk | wv] along the last axis, fp32
    wqkv_d = nc.dram_tensor("wqkv", [P, 2, 3 * HD], f32, kind="ExternalInput").ap()
    wp_d = nc.dram_tensor("wp", [HD, C], adt if mm == "f16" else f32,
                          kind="ExternalInput").ap()
    aux_d = nc.dram_tensor("aux", [P, NAUX], f32, kind="ExternalInput").ap()
    gbc_d = nc.dram_tensor("gbc", [GROUPS, 2, P], f32, kind="ExternalInput").ap()
    out_d = nc.dram_tensor("out", [P, 2, HW], f32, kind="ExternalOutput").ap()
    den_d = nc.dram_tensor("den", [NIB, IB], f32, kind="ExternalOutput").ap()
    bve_d = nc.dram_tensor("bve", [HD, 1], f32, kind="ExternalOutput").ap()

    with tile.TileContext(nc) as tc:
        with (
            tc.tile_pool(name="const", bufs=1) as const,
            tc.tile_pool(name="big", bufs=1) as big,
            tc.tile_pool(name="tmp", bufs=3) as tmp,
            tc.tile_pool(name="pt", bufs=4) as ptp,
            tc.tile_pool(name="oh", bufs=2) as ohp,
            tc.tile_pool(name="ostage", bufs=3) as ostage,
            tc.tile_pool(name="ps_st", bufs=2, space="PSUM") as ps_st,
            tc.tile_pool(name="ps_pv", bufs=1, space="PSUM") as ps_pv,
            tc.tile_pool(name="ps_sm", bufs=2, space="PSUM") as ps_sm,
        ):
            eps_sb = const.tile([GROUPS, 1], f32)
            nc.vector.memset(eps_sb, EPS)
            ones_sb = const.tile([P, 1], f32)
            nc.vector.memset(ones_sb, 1.0)
            # Touch Exp immediately so walrus's single ACT table load runs
            # during the DMA-in phase.
            warm_sb = tmp.tile([GROUPS, 1], f32, tag="warm")
            nc.scalar.activation(out=warm_sb, in_=eps_sb, func=AF.Exp, scale=1.0)

            # ---- x first (chunked so stats start early); per-channel
            # moment accumulation split across DVE (po=0, bn_stats) and
            # ScalarE (po=1, Identity/Square with accum_out) ----
            x16 = big.tile([P, 2, HW], adt)
            stats0 = tmp.tile([P, 8, 6], f32, tag="bnstats0", name="stats0")
            psums1 = tmp.tile([P, 4], f32, tag="psums1", name="psums1")
            scratch = tmp.tile([P, HW // 2], adt, tag="scratch", name="scratch")
            for c in range(2):
                cs = slice(c * (HW // 2), (c + 1) * (HW // 2))
                nc.sync.dma_start(x16[:, 0, cs], x_d[:, 0, cs])
                for s in range(4 * c, 4 * c + 4):
                    nc.vector.bn_stats(
                        out=stats0[:, s, :],
                        in_=x16[:, 0, s * 512:(s + 1) * 512],
                    )
                nc.sync.dma_start(x16[:, 1, cs], x_d[:, 1, cs])
                nc.scalar.activation(out=scratch, in_=x16[:, 1, cs],
                                     func=AF.Identity,
                                     accum_out=psums1[:, c:c + 1])
                nc.scalar.activation(out=scratch, in_=x16[:, 1, cs],
                                     func=AF.Square,
                                     accum_out=psums1[:, 2 + c:3 + c])
            wqkv_sb = const.tile([P, 2, 3 * HD], f32)
            nc.sync.dma_start(wqkv_sb, wqkv_d)
            aux_sb = const.tile([P, NAUX], f32)
            nc.sync.dma_start(aux_sb, aux_d)
            gbc_sb = const.tile([GROUPS, 2, P], f32)
            nc.sync.dma_start(gbc_sb, gbc_d)
            wp_sb = const.tile([HD, C], adt if mm == "f16" else f32)
            nc.sync.dma_start(wp_sb, wp_d)

            # po=0: [mean_c, E[x^2]_c] from bn_aggr; po=1: raw [sum, sumsq]
            # (its gmat carries the extra 1/4096)
            mv0 = const.tile([P, 2], f32, tag="mv0")
            nc.vector.bn_aggr(out=mv0, in_=stats0)
            msq = tmp.tile([P, 1], f32, tag="msq")
            nc.vector.tensor_mul(msq, mv0[:, 0:1], mv0[:, 0:1])
            nc.vector.tensor_add(mv0[:, 1:2], mv0[:, 1:2], msq)
            mv1 = const.tile([P, 2], f32, tag="mv1")
            nc.vector.tensor_add(mv1[:, 0:1], psums1[:, 0:1], psums1[:, 1:2])
            nc.vector.tensor_add(mv1[:, 1:2], psums1[:, 2:3], psums1[:, 3:4])
            mv = [mv0, mv1]

            # group-level [mean_g, E[x^2]_g] via indicator matmul (values 1/8)
            gst_ps = ps_sm.tile([GROUPS, 2], f32, tag="small")
            nc.tensor.matmul(gst_ps, lhsT=aux_sb[:, 7:7 + GROUPS], rhs=mv[0],
                             start=True, stop=False)
            nc.tensor.matmul(gst_ps, lhsT=aux_sb[:, 7 + GROUPS:7 + 2 * GROUPS],
                             rhs=mv[1], start=False, stop=True)
            gst = const.tile([GROUPS, 2], f32)
            nc.vector.tensor_copy(gst, gst_ps)

            # var_g = E[x^2]_g - mean_g^2 + eps; rs = rsqrt(var) via the
            # bit-trick seed + 3 Newton iterations, all on the DVE
            varg = tmp.tile([GROUPS, 1], f32, tag="varg")
            nc.vector.tensor_mul(varg, gst[:, 0:1], gst[:, 0:1])
            nc.vector.tensor_sub(varg, gst[:, 1:2], varg)
            nc.vector.tensor_scalar_add(varg, varg, float(EPS))
            vhalf = tmp.tile([GROUPS, 1], f32, tag="vhalf")
            nc.vector.tensor_scalar_mul(vhalf, varg, 0.5)
            st = const.tile([GROUPS, 2], f32)  # [rs_g, -mu_g*rs_g]
            y = st[:, 0:1]
            i32 = mybir.dt.int32
            nc.vector.tensor_scalar(
                out=y.bitcast(i32), in0=varg.bitcast(i32),
                scalar1=1, scalar2=None, op0=OP.logical_shift_right,
            )
            nc.vector.tensor_scalar(
                out=y.bitcast(i32), in0=y.bitcast(i32),
                scalar1=-1, scalar2=0x5F3759DF, op0=OP.mult, op1=OP.add,
            )
            tnr = tmp.tile([GROUPS, 1], f32, tag="tnr")
            for _ in range(3):
                nc.vector.tensor_mul(tnr, y, y)
                nc.vector.tensor_mul(tnr, tnr, vhalf)
                nc.vector.tensor_scalar(
                    out=tnr, in0=tnr, scalar1=-1.0, scalar2=1.5,
                    op0=OP.mult, op1=OP.add,
                )
                nc.vector.tensor_mul(y, y, tnr)
            nc.vector.tensor_mul(st[:, 1:2], gst[:, 0:1], st[:, 0:1])
            nc.vector.tensor_scalar_mul(st[:, 1:2], st[:, 1:2], -1.0)

            # per-channel [s_c, t_c] (gn weight/bias applied)
            sca = []
            for po in range(2):
                stc_ps = ps_sm.tile([P, 2], f32, tag="small")
                nc.tensor.matmul(stc_ps, lhsT=gbc_sb[:, po, :], rhs=st,
                                 start=True, stop=True)
                sc = const.tile([P, 2], f32, tag=f"sca{po}")
                nc.vector.tensor_mul(sc[:, 0:1], stc_ps[:, 0:1],
                                     aux_sb[:, 3 + po:4 + po])
                nc.vector.tensor_mul(sc[:, 1:2], stc_ps[:, 1:2],
                                     aux_sb[:, 3 + po:4 + po])
                nc.vector.tensor_add(sc[:, 1:2], sc[:, 1:2],
                                     aux_sb[:, 5 + po:6 + po])
                sca.append(sc)

            # scaled fp16 qkv weights: W'[.,c] = W[.,c] * s_c
            wsc = const.tile([P, 2, 3 * HD], adt)
            for po in range(2):
                nc.vector.tensor_scalar_mul(wsc[:, po, :], wqkv_sb[:, po, :],
                                            sca[po][:, 0:1])

            # effective biases: b' = W @ t + b_raw.
            # q and k adj land on both partition halves via col-packed N=1
            # matmuls; v adj on rows 0:64 only.
            bqe = const.tile([P, 1], f32)
            bke = const.tile([P, 1], f32)
            bve = const.tile([P, 1], f32)
            for (wofs, dst, rawcol) in ((0, bqe, 0), (HD, bke, 1),
                                        (2 * HD, bve, 2)):
                bp = ps_sm.tile([P, 1], f32, tag="small", name="bp")
                halves = (0, 1) if rawcol < 2 else (0,)
                for half in halves:
                    hs = slice(half * HD, (half + 1) * HD)
                    nc.tensor.matmul(bp[hs, :],
                                     lhsT=wqkv_sb[:, 0, wofs:wofs + HD],
                                     rhs=sca[0][:, 1:2], start=True, stop=False)
                    nc.tensor.matmul(bp[hs, :],
                                     lhsT=wqkv_sb[:, 1, wofs:wofs + HD],
                                     rhs=sca[1][:, 1:2], start=False, stop=True)
                nc.vector.tensor_add(dst[:HD * len(halves), :],
                                     bp[:HD * len(halves), :],
                                     aux_sb[:HD * len(halves),
                                            rawcol:rawcol + 1])
            nc.sync.dma_start(bve_d, bve[0:HD, :])

            # ---- qkv on x16; q and k duplicated onto partitions 64..127
            # via col-packed matmuls (concurrent on the PE array) ----
            qq_sb = big.tile([P, HW], adt)
            kk_sb = big.tile([P, HW], adt)
            vt_sb = big.tile([P, NJC, HD + 1], adt)
            nc.vector.tensor_copy(
                vt_sb[:, :, HD:HD + 1],
                ones_sb[:, None, :].to_broadcast([P, NJC, 1]),
            )
            for n in range(HW // 512):
                ns = slice(n * 512, (n + 1) * 512)
                for (wofs, bsb, dst) in ((0, bqe, qq_sb), (HD, bke, kk_sb)):
                    qp = ps_sm.tile([P, 512], f32, tag="small", name="qp")
                    for half in range(2):
                        hs = slice(half * HD, (half + 1) * HD)
                        nc.tensor.matmul(qp[hs, :],
                                         lhsT=wsc[:, 0, wofs:wofs + HD],
                                         rhs=x16[:, 0, ns],
                                         start=True, stop=False)
                        nc.tensor.matmul(qp[hs, :],
                                         lhsT=wsc[:, 1, wofs:wofs + HD],
                                         rhs=x16[:, 1, ns],
                                         start=False, stop=True)
                    nc.vector.tensor_scalar_add(dst[:, ns], qp, bsb)
            # v^T directly: [positions, head_dim], chunked by 128 positions
            for jc in range(NJC):
                js = slice(jc * P, (jc + 1) * P)
                vp = ps_sm.tile([P, HD], f32, tag="small", name="vp")
                nc.tensor.matmul(vp, lhsT=x16[:, 0, js],
                                 rhs=wsc[:, 0, 2 * HD:3 * HD],
                                 start=True, stop=False)
                nc.tensor.matmul(vp, lhsT=x16[:, 1, js],
                                 rhs=wsc[:, 1, 2 * HD:3 * HD],
                                 start=False, stop=True)
                nc.vector.tensor_copy(vt_sb[:, jc, 0:HD], vp)

            # ---- attention + proj, blocked over queries. The previous
            # block's epilogue is emitted after the first two exp's of the
            # next block so the ScalarE never stalls at block boundaries ----
            SC = float(1.0 / np.sqrt(HD))
            pend = []

            def emit_epilogue():
                if not pend:
                    return
                ib0, pv0 = pend.pop()
                ibs0 = ib0 * IB
                oh16 = ohp.tile([HD, IB], adt, tag="oh16", name="oh16")
                nc.vector.tensor_copy(oh16, pv0[0:HD, :])
                den_sb = ohp.tile([1, IB], f32, tag="den", name="den_sb")
                nc.vector.tensor_copy(den_sb, pv0[HD:HD + 1, :])
                nc.sync.dma_start(den_d[ib0:ib0 + 1, :], den_sb)
                for mt in range(2):
                    for n2 in range(IB // 512):
                        pp = ps_sm.tile([P, 512], f32, tag="small", name="pp")
                        nc.tensor.matmul(
                            pp,
                            lhsT=wp_sb[:, mt * P:(mt + 1) * P],
                            rhs=oh16[:, n2 * 512:(n2 + 1) * 512],
                            start=True, stop=True,
                        )
                        sg = ostage.tile([P, 512], f32, tag="ostage", name="sg")
                        nc.vector.tensor_copy(sg, pp)
                        nc.sync.dma_start(
                            out_d[:, mt, ibs0 + n2 * 512: ibs0 + (n2 + 1) * 512],
                            sg)

            for ib in range(NIB):
                ibs = ib * IB
                pts = {}
                # S + exp for the first two key-chunks before the previous
                # block's epilogue claims the PE
                for jc in range(2):
                    st_ps = ps_st.tile([P, IB], f32, tag="st", name="st_ps")
                    nc.tensor.matmul(
                        st_ps[:, 0:512],
                        lhsT=kk_sb[0:HD, jc * P:(jc + 1) * P],
                        rhs=qq_sb[0:HD, ibs: ibs + 512],
                        start=True, stop=True,
                    )
                    nc.tensor.matmul(
                        st_ps[:, 512:1024],
                        lhsT=kk_sb[HD:P, jc * P:(jc + 1) * P],
                        rhs=qq_sb[HD:P, ibs + 512: ibs + 1024],
                        start=True, stop=True,
                    )
                    pt = ptp.tile([P, IB], adt, tag="pt", name="pt")
                    nc.scalar.activation(out=pt, in_=st_ps, func=AF.Exp,
                                         scale=SC)
                    pts[jc] = pt
                emit_epilogue()
                pv_ps = ps_pv.tile([HD + 1, IB], f32, tag="pv", name="pv_ps")
                for jc in range(NJC):
                    if jc in pts:
                        pt = pts.pop(jc)
                    else:
                        st_ps = ps_st.tile([P, IB], f32, tag="st", name="st_ps")
                        nc.tensor.matmul(
                            st_ps[:, 0:512],
                            lhsT=kk_sb[0:HD, jc * P:(jc + 1) * P],
                            rhs=qq_sb[0:HD, ibs: ibs + 512],
                            start=True, stop=True,
                        )
                        nc.tensor.matmul(
                            st_ps[:, 512:1024],
                            lhsT=kk_sb[HD:P, jc * P:(jc + 1) * P],
                            rhs=qq_sb[HD:P, ibs + 512: ibs + 1024],
                            start=True, stop=True,
                        )
                        pt = ptp.tile([P, IB], adt, tag="pt", name="pt")
                        nc.scalar.activation(out=pt, in_=st_ps, func=AF.Exp,
                                             scale=SC)
                    for n2 in range(IB // 512):
                        nc.tensor.matmul(
                            pv_ps[:, n2 * 512:(n2 + 1) * 512],
                            lhsT=vt_sb[:, jc, :],
                            rhs=pt[:, n2 * 512:(n2 + 1) * 512],
                            start=(jc == 0), stop=(jc == NJC - 1),
                        )
                pend.append((ib, pv_ps))
            emit_epilogue()
    nc.compile()
    return nc


def get_module(mm=MM_MODE):
    if mm not in _module_cache:
        _module_cache[mm] = _build_module(mm)
    return _module_cache[mm]


def _group_mats():
    gmat = np.zeros((P, 2, GROUPS), np.float32)
    gbc = np.zeros((GROUPS, 2, P), np.float32)
    for po in range(2):
        for pi in range(P):
            g = (po * P + pi) // 8
            gmat[pi, po, g] = 1.0 / 8.0
            gbc[g, po, pi] = 1.0
    return gmat, gbc


def make_in_maps(x, gn_weight, gn_bias, qkv_weight, qkv_bias,
                 proj_weight=None, mm=None):
    mm = mm or MM_MODE
    wp_np = np.float16 if mm == "f16" else np.float32
    x = np.asarray(x, np.float32)
    gn_weight = np.asarray(gn_weight, np.float32)
    gn_bias = np.asarray(gn_bias, np.float32)
    qkv_weight = np.asarray(qkv_weight, np.float32)
    qkv_bias = np.asarray(qkv_bias, np.float32)
    gmat, gbc = _group_mats()
    gnw = np.ascontiguousarray(gn_weight.reshape(2, P).T)   # [128, 2]
    gnb = np.ascontiguousarray(gn_bias.reshape(2, P).T)

    def wslice(row0):
        w = qkv_weight[row0:row0 + HD, :]            # [64, 256]
        return w.T.reshape(2, P, HD).transpose(1, 0, 2)

    def bias2(off):
        b = qkv_bias[off:off + HD].reshape(HD, 1)
        return np.vstack([b, b])

    wps = [None] * NH
    if proj_weight is not None:
        pw = np.asarray(proj_weight, np.float32)
        wps = [np.ascontiguousarray(
            pw[:, h * HD:(h + 1) * HD].T.astype(wp_np)) for h in range(NH)]

    gmat = gmat.copy()
    gmat[:, 1, :] /= HW  # po=1 stats arrive as raw sums
    in_maps = []
    for b in range(B):
        xt = np.ascontiguousarray(
            x[b].reshape(2, P, HW).transpose(1, 0, 2).astype(wp_np))
        for h in range(NH):
            wqkv = np.concatenate(
                [wslice(h * HD), wslice(C + h * HD), wslice(2 * C + h * HD)],
                axis=2).astype(np.float32)
            bv = np.zeros((P, 1), np.float32)
            bv[0:HD, 0] = qkv_bias[2 * C + h * HD: 2 * C + (h + 1) * HD]
            aux = np.concatenate(
                [bias2(h * HD), bias2(C + h * HD), bv,
                 gnw[:, 0:1], gnw[:, 1:2], gnb[:, 0:1], gnb[:, 1:2],
                 gmat[:, 0, :], gmat[:, 1, :]], axis=1).astype(np.float32)
            in_maps.append({
                "x": xt,
                "wqkv": np.ascontiguousarray(wqkv),
                "wp": wps[h],
                "aux": np.ascontiguousarray(aux),
                "gbc": gbc,
            })
    return in_maps


def combine_outputs(results, x, proj_weight, proj_bias):
    """results: 8 dicts with 'out' [128,2,HW], 'den' [NIB,IB], 'bve' [HD,1]."""
    x = np.asarray(x, np.float32)
    proj_weight = np.asarray(proj_weight, np.float32)
    proj_bias = np.asarray(proj_bias, np.float32)
    y = np.empty((B, C, H, W), np.float32)
    for b in range(B):
        acc = x[b].reshape(C, HW) + proj_bias[:, None]
        for h in range(NH):
            r = results[b * NH + h]
            part = np.asarray(r["out"]).transpose(1, 0, 2).reshape(C, HW)
            den = np.asarray(r["den"]).reshape(HW)
            bve = np.asarray(r["bve"]).reshape(HD)
            ch = proj_weight[:, h * HD:(h + 1) * HD] @ bve
            acc = acc + part / den[None, :] + ch[:, None]
        y[b] = acc.reshape(C, H, W)
    return y


def kernel(x, gn_weight, gn_bias, qkv_weight, qkv_bias, proj_weight, proj_bias):
    from concourse.bass_utils import run_bass_kernel_spmd

    nc = get_module()
    in_maps = make_in_maps(x, gn_weight, gn_bias, qkv_weight, qkv_bias,
                           proj_weight=proj_weight)
    res = run_bass_kernel_spmd(nc, in_maps, core_ids=list(range(NCORES)))
    return combine_outputs(res.results, x, proj_weight, proj_bias)


# revision 32
# speedup vs baseline: 1.2510x; 1.0143x over previous
"""Trainium2 Bass kernel for nn_Attention2d.

Computation: GroupNorm(32 groups) -> 1x1 qkv conv -> 4-head attention over
H*W=4096 positions -> 1x1 proj conv -> residual add.

Sharding: one (batch, head) pair per NeuronCore (B=2 x NH=4 = 8 cores).
Each core:
  - GroupNorm stats of its batch slice; the affine normalization is folded
    into the qkv weights (W' = W*s per input channel) and effective biases
    (b' = W@t + b), so the x-sized tensor is only cast to fp16 once
  - its head's q/k (with effective bias) and v^T (bias exported to host)
  - S^T = k^T q in [keys-on-partitions, queries-on-free] layout
    (no max-subtraction: |S/8| <~ 6 so exp is safe in fp32)
  - P^T = exp(S^T/8); PV via matmul with lhsT = [v^T | ones]  -> the ones
    column yields the softmax denominators for free (row 64 of the output)
  - proj partial = Wp[:, head]^T @ PV_raw  (un-normalized)
Host: out[b] = x[b] + proj_bias + sum_h (partial_h/denom_h + Wp_h @ bve_h)
(the softmax normalization and the constant v-bias commute through proj).

PE-array packing: the S matmuls contract over only hd=64 partitions, so q and
k are duplicated onto partitions 64..127 (by col-packed qkv matmuls that cost
no extra PE time) and each S^T tile is computed as two concurrent matmuls on
row-groups (0,0) and (64,0).

Matmul dtypes: qkv/attention/proj matmuls use float16 operands (1 cycle/row,
fast weight loads); GroupNorm matmuls, softmax denominators and all
reductions stay fp32. GroupNorm's rsqrt runs on the DVE (bit-trick seed +
Newton) so the ScalarE keeps a single Exp table set for the whole kernel.
"""

import numpy as np

B, C, H, W = 2, 256, 64, 64
HW = H * W           # 4096
GROUPS = 32
NH = 4
HD = C // NH         # 64
EPS = 1e-5
P = 128
IB = 1024            # query block (PSUM-sized)
NIB = HW // IB       # 4
NJC = HW // P        # 32 key chunks
NCORES = B * NH

# "f32": exact fp32 everywhere (slow). "f32r": float32r operands
# (TF32-like rounding, ~3 cycles/row on HW). "f16": float16 operands.
MM_MODE = "f16"

# aux column layout: 0 bq2, 1 bk2, 2 bv (rows 0:64), 3+po gnw, 5+po gnb,
# 7+32*po gmat
NAUX = 7 + 2 * GROUPS

_module_cache = {}


def _build_module(mm=MM_MODE):
    import concourse.bacc as bacc
    import concourse.tile as tile
    import concourse.mybir as mybir

    dt = mybir.dt
    f32 = dt.float32
    AF = mybir.ActivationFunctionType
    OP = mybir.AluOpType
    if mm == "f32":
        adt = f32
    elif mm == "f32r":
        adt = dt.float32r
    elif mm == "f16":
        adt = dt.float16
    else:
        raise ValueError(mm)

    nc = bacc.Bacc(trn_type="TRN2", target_bir_lowering=False, debug=False)

    # ---- DRAM I/O (per-core tensors; host prepares layouts) ----
    # channel layout everywhere: c = po*128 + pi  ->  [pi, po, ...]
    # x arrives already cast to the attention dtype (host-side cast); the
    # GroupNorm statistics absorb the rounding (it averages out over 32k
    # elements per group).
    x_d = nc.dram_tensor("x", [P, 2, HW], adt, kind="ExternalInput").ap()
    # packed raw qkv weight slices: [wq | wk | wv] along the last axis, fp32
    wqkv_d = nc.dram_tensor("wqkv", [P, 2, 3 * HD], f32, kind="ExternalInput").ap()
    wp_d = nc.dram_tensor("wp", [HD, C], adt if mm == "f16" else f32,
                          kind="ExternalInput").ap()
    aux_d = nc.dram_tensor("aux", [P, NAUX], f32, kind="ExternalInput").ap()
    gbc_d = nc.dram_tensor("gbc", [GROUPS, 2, P], f32, kind="ExternalInput").ap()
    out_d = nc.dram_tensor("out", [P, 2, HW], f32, kind="ExternalOutput").ap()
    den_d = nc.dram_tensor("den", [NIB, IB], f32, kind="ExternalOutput").ap()
    bve_d = nc.dram_tensor("bve", [HD, 1], f32, kind="ExternalOutput").ap()

    with tile.TileContext(nc) as tc:
        with (
            tc.tile_pool(name="const", bufs=1) as const,
            tc.tile_pool(name="big", bufs=1) as big,
            tc.tile_pool(name="tmp", bufs=3) as tmp,
            tc.tile_pool(name="pt", bufs=4) as ptp,
            tc.tile_pool(name="oh", bufs=2) as ohp,
            tc.tile_pool(name="ostage", bufs=3) as ostage,
            tc.tile_pool(name="ps_st", bufs=2, space="PSUM") as ps_st,
            tc.tile_pool(name="ps_pv", bufs=1, space="PSUM") as ps_pv,
            tc.tile_pool(name="ps_sm", bufs=2, space="PSUM") as ps_sm,
        ):
            eps_sb = const.tile([GROUPS, 1], f32)
            nc.vector.memset(eps_sb, EPS)
            ones_sb = const.tile([P, 1], f32)
            nc.vector.memset(ones_sb, 1.0)
            # Touch Exp immediately so walrus's single ACT table load runs
            # during the DMA-in phase.
            warm_sb = tmp.tile([GROUPS, 1], f32, tag="warm")
            nc.scalar.activation(out=warm_sb, in_=eps_sb, func=AF.Exp, scale=1.0)

            # ---- x first (chunked so stats start early); per-channel
            # moment accumulation split across DVE (po=0, bn_stats) and
            # ScalarE (po=1, Identity/Square with accum_out) ----
            x16 = big.tile([P, 2, HW], adt)
            stats0 = tmp.tile([P, 8, 6], f32, tag="bnstats0", name="stats0")
            psums1 = tmp.tile([P, 4], f32, tag="psums1", name="psums1")
            scratch = tmp.tile([P, HW // 2], adt, tag="scratch", name="scratch")
            for c in range(2):
                cs = slice(c * (HW // 2), (c + 1) * (HW // 2))
                # po=1 via the ScalarE HWDGE queue (parallel issue with po=0)
                nc.scalar.dma_start(x16[:, 1, cs], x_d[:, 1, cs])
                nc.sync.dma_start(x16[:, 0, cs], x_d[:, 0, cs])
                nc.scalar.activation(out=scratch, in_=x16[:, 1, cs],
                                     func=AF.Identity,
                                     accum_out=psums1[:, c:c + 1])
                nc.scalar.activation(out=scratch, in_=x16[:, 1, cs],
                                     func=AF.Square,
                                     accum_out=psums1[:, 2 + c:3 + c])
                for s in range(4 * c, 4 * c + 4):
                    nc.vector.bn_stats(
                        out=stats0[:, s, :],
                        in_=x16[:, 0, s * 512:(s + 1) * 512],
                    )
            wqkv_sb = const.tile([P, 2, 3 * HD], f32)
            nc.sync.dma_start(wqkv_sb, wqkv_d)
            aux_sb = const.tile([P, NAUX], f32)
            nc.sync.dma_start(aux_sb, aux_d)
            gbc_sb = const.tile([GROUPS, 2, P], f32)
            nc.sync.dma_start(gbc_sb, gbc_d)
            wp_sb = const.tile([HD, C], adt if mm == "f16" else f32)
            nc.sync.dma_start(wp_sb, wp_d)

            # po=0: [mean_c, E[x^2]_c] from bn_aggr; po=1: raw [sum, sumsq]
            # (its gmat carries the extra 1/4096)
            mv0 = const.tile([P, 2], f32, tag="mv0")
            nc.vector.bn_aggr(out=mv0, in_=stats0)
            msq = tmp.tile([P, 1], f32, tag="msq")
            nc.vector.tensor_mul(msq, mv0[:, 0:1], mv0[:, 0:1])
            nc.vector.tensor_add(mv0[:, 1:2], mv0[:, 1:2], msq)
            mv1 = const.tile([P, 2], f32, tag="mv1")
            nc.vector.tensor_add(mv1[:, 0:1], psums1[:, 0:1], psums1[:, 1:2])
            nc.vector.tensor_add(mv1[:, 1:2], psums1[:, 2:3], psums1[:, 3:4])
            mv = [mv0, mv1]

            # group-level [mean_g, E[x^2]_g] via indicator matmul (values 1/8)
            gst_ps = ps_sm.tile([GROUPS, 2], f32, tag="small")
            nc.tensor.matmul(gst_ps, lhsT=aux_sb[:, 7:7 + GROUPS], rhs=mv[0],
                             start=True, stop=False)
            nc.tensor.matmul(gst_ps, lhsT=aux_sb[:, 7 + GROUPS:7 + 2 * GROUPS],
                             rhs=mv[1], start=False, stop=True)
            gst = const.tile([GROUPS, 2], f32)
            nc.vector.tensor_copy(gst, gst_ps)

            # var_g = E[x^2]_g - mean_g^2 + eps; rs = rsqrt(var) via the
            # bit-trick seed + 2 Newton iterations, all on the DVE
            varg = tmp.tile([GROUPS, 1], f32, tag="varg")
            nc.vector.tensor_mul(varg, gst[:, 0:1], gst[:, 0:1])
            nc.vector.tensor_sub(varg, gst[:, 1:2], varg)
            nc.vector.tensor_scalar_add(varg, varg, float(EPS))
            st = const.tile([GROUPS, 2], f32)  # [rs_g, -mu_g*rs_g]
            y = st[:, 0:1]
            i32 = mybir.dt.int32
            nc.vector.tensor_scalar(
                out=y.bitcast(i32), in0=varg.bitcast(i32),
                scalar1=1, scalar2=None, op0=OP.logical_shift_right,
            )
            nc.vector.tensor_scalar(
                out=y.bitcast(i32), in0=y.bitcast(i32),
                scalar1=-1, scalar2=0x5F3759DF, op0=OP.mult, op1=OP.add,
            )
            tnr = tmp.tile([GROUPS, 1], f32, tag="tnr")
            for it in range(3):
                nc.vector.tensor_mul(tnr, y, y)
                # tnr = (tnr * 0.5) * varg
                nc.vector.scalar_tensor_tensor(
                    out=tnr, in0=tnr, scalar=0.5, in1=varg,
                    op0=OP.mult, op1=OP.mult,
                )
                nc.vector.tensor_scalar(
                    out=tnr, in0=tnr, scalar1=-1.0, scalar2=1.5,
                    op0=OP.mult, op1=OP.add,
                )
                nc.vector.tensor_mul(y, y, tnr)
            nc.vector.tensor_mul(st[:, 1:2], gst[:, 0:1], st[:, 0:1])
            nc.vector.tensor_scalar_mul(st[:, 1:2], st[:, 1:2], -1.0)

            # per-channel [s_c, t_c]: gbc carries gn_weight, and W@gn_bias is
            # folded into the host-side raw biases, so the broadcast matmul
            # output is used directly
            sca = []
            for po in range(2):
                stc_ps = ps_sm.tile([P, 2], f32, tag="small")
                nc.tensor.matmul(stc_ps, lhsT=gbc_sb[:, po, :], rhs=st,
                                 start=True, stop=True)
                sc = const.tile([P, 2], f32, tag=f"sca{po}")
                nc.vector.tensor_copy(sc, stc_ps)
                sca.append(sc)

            # scaled fp16 qkv weights: W'[.,c] = W[.,c] * s_c
            wsc = const.tile([P, 2, 3 * HD], adt)
            for po in range(2):
                nc.vector.tensor_scalar_mul(wsc[:, po, :], wqkv_sb[:, po, :],
                                            sca[po][:, 0:1])

            # effective biases: b' = W @ t + b_raw.
            # q and k adj land on both partition halves via col-packed N=1
            # matmuls; v adj on rows 0:64 only.
            bqe = const.tile([P, 1], f32)
            bke = const.tile([P, 1], f32)
            bve = const.tile([P, 1], f32)
            for (wofs, dst, rawcol) in ((0, bqe, 0), (HD, bke, 1),
                                        (2 * HD, bve, 2)):
                bp = ps_sm.tile([P, 1], f32, tag="small", name="bp")
                halves = (0, 1) if rawcol < 2 else (0,)
                for half in halves:
                    hs = slice(half * HD, (half + 1) * HD)
                    nc.tensor.matmul(bp[hs, :],
                                     lhsT=wqkv_sb[:, 0, wofs:wofs + HD],
                                     rhs=sca[0][:, 1:2], start=True, stop=False)
                    nc.tensor.matmul(bp[hs, :],
                                     lhsT=wqkv_sb[:, 1, wofs:wofs + HD],
                                     rhs=sca[1][:, 1:2], start=False, stop=True)
                nc.vector.tensor_add(dst[:HD * len(halves), :],
                                     bp[:HD * len(halves), :],
                                     aux_sb[:HD * len(halves),
                                            rawcol:rawcol + 1])
            nc.sync.dma_start(bve_d, bve[0:HD, :])

            # ---- qkv on x16; q and k duplicated onto partitions 64..127
            # via col-packed matmuls (concurrent on the PE array) ----
            qq_sb = big.tile([P, HW], adt)
            kk_sb = big.tile([P, HW], adt)
            vt_sb = big.tile([P, NJC, HD + 1], adt)
            nc.vector.tensor_copy(
                vt_sb[:, :, HD:HD + 1],
                ones_sb[:, None, :].to_broadcast([P, NJC, 1]),
            )
            for n in range(HW // 512):
                ns = slice(n * 512, (n + 1) * 512)
                for (wofs, bsb, dst) in ((0, bqe, qq_sb), (HD, bke, kk_sb)):
                    qp = ps_sm.tile([P, 512], f32, tag="small", name="qp")
                    for half in range(2):
                        hs = slice(half * HD, (half + 1) * HD)
                        nc.tensor.matmul(qp[hs, :],
                                         lhsT=wsc[:, 0, wofs:wofs + HD],
                                         rhs=x16[:, 0, ns],
                                         start=True, stop=False)
                        nc.tensor.matmul(qp[hs, :],
                                         lhsT=wsc[:, 1, wofs:wofs + HD],
                                         rhs=x16[:, 1, ns],
                                         start=False, stop=True)
                    nc.vector.tensor_scalar_add(dst[:, ns], qp, bsb)
            # v^T directly: [positions, head_dim], chunked by 128 positions
            for jc in range(NJC):
                js = slice(jc * P, (jc + 1) * P)
                vp = ps_sm.tile([P, HD], f32, tag="small", name="vp")
                nc.tensor.matmul(vp, lhsT=x16[:, 0, js],
                                 rhs=wsc[:, 0, 2 * HD:3 * HD],
                                 start=True, stop=False)
                nc.tensor.matmul(vp, lhsT=x16[:, 1, js],
                                 rhs=wsc[:, 1, 2 * HD:3 * HD],
                                 start=False, stop=True)
                nc.vector.tensor_copy(vt_sb[:, jc, 0:HD], vp)

            # ---- attention + proj, blocked over queries. The previous
            # block's epilogue is emitted after the first two exp's of the
            # next block so the ScalarE never stalls at block boundaries ----
            SC = float(1.0 / np.sqrt(HD))
            pend = []

            def emit_epilogue():
                if not pend:
                    return
                ib0, pv0 = pend.pop()
                ibs0 = ib0 * IB
                oh16 = ohp.tile([HD, IB], adt, tag="oh16", name="oh16")
                nc.vector.tensor_copy(oh16, pv0[0:HD, :])
                den_sb = ohp.tile([1, IB], f32, tag="den", name="den_sb")
                nc.vector.tensor_copy(den_sb, pv0[HD:HD + 1, :])
                nc.sync.dma_start(den_d[ib0:ib0 + 1, :], den_sb)
                for mt in range(2):
                    for n2 in range(IB // 512):
                        pp = ps_sm.tile([P, 512], f32, tag="small", name="pp")
                        nc.tensor.matmul(
                            pp,
                            lhsT=wp_sb[:, mt * P:(mt + 1) * P],
                            rhs=oh16[:, n2 * 512:(n2 + 1) * 512],
                            start=True, stop=True,
                        )
                        sg = ostage.tile([P, 512], f32, tag="ostage", name="sg")
                        nc.vector.tensor_copy(sg, pp)
                        nc.sync.dma_start(
                            out_d[:, mt, ibs0 + n2 * 512: ibs0 + (n2 + 1) * 512],
                            sg)

            for ib in range(NIB):
                ibs = ib * IB
                pts = {}
                # S + exp for the first two key-chunks before the previous
                # block's epilogue claims the PE
                for jc in range(2):
                    st_ps = ps_st.tile([P, IB], f32, tag="st", name="st_ps")
                    nc.tensor.matmul(
                        st_ps[:, 0:512],
                        lhsT=kk_sb[0:HD, jc * P:(jc + 1) * P],
                        rhs=qq_sb[0:HD, ibs: ibs + 512],
                        start=True, stop=True,
                    )
                    nc.tensor.matmul(
                        st_ps[:, 512:1024],
                        lhsT=kk_sb[HD:P, jc * P:(jc + 1) * P],
                        rhs=qq_sb[HD:P, ibs + 512: ibs + 1024],
                        start=True, stop=True,
                    )
                    pt = ptp.tile([P, IB], adt, tag="pt", name="pt")
                    nc.scalar.activation(out=pt, in_=st_ps, func=AF.Exp,
                                         scale=SC)
                    pts[jc] = pt
                emit_epilogue()
                pv_ps = ps_pv.tile([HD + 1, IB], f32, tag="pv", name="pv_ps")
                for jc in range(NJC):
                    if jc in pts:
                        pt = pts.pop(jc)
                    else:
                        st_ps = ps_st.tile([P, IB], f32, tag="st", name="st_ps")
                        nc.tensor.matmul(
                            st_ps[:, 0:512],
                            lhsT=kk_sb[0:HD, jc * P:(jc + 1) * P],
                            rhs=qq_sb[0:HD, ibs: ibs + 512],
                            start=True, stop=True,
                        )
                        nc.tensor.matmul(
                            st_ps[:, 512:1024],
                            lhsT=kk_sb[HD:P, jc * P:(jc + 1) * P],
                            rhs=qq_sb[HD:P, ibs + 512: ibs + 1024],
                            start=True, stop=True,
                        )
                        pt = ptp.tile([P, IB], adt, tag="pt", name="pt")
                        nc.scalar.activation(out=pt, in_=st_ps, func=AF.Exp,
                                             scale=SC)
                    for n2 in range(IB // 512):
                        nc.tensor.matmul(
                            pv_ps[:, n2 * 512:(n2 + 1) * 512],
                            lhsT=vt_sb[:, jc, :],
                            rhs=pt[:, n2 * 512:(n2 + 1) * 512],
                            start=(jc == 0), stop=(jc == NJC - 1),
                        )
                pend.append((ib, pv_ps))
            emit_epilogue()
    nc.compile()
    return nc


def get_module(mm=MM_MODE):
    if mm not in _module_cache:
        _module_cache[mm] = _build_module(mm)
    return _module_cache[mm]


def _group_mats(gn_weight):
    gmat = np.zeros((P, 2, GROUPS), np.float32)
    gbc = np.zeros((GROUPS, 2, P), np.float32)
    for po in range(2):
        for pi in range(P):
            c = po * P + pi
            g = c // 8
            gmat[pi, po, g] = 1.0 / 8.0
            gbc[g, po, pi] = gn_weight[c]
    return gmat, gbc


def make_in_maps(x, gn_weight, gn_bias, qkv_weight, qkv_bias,
                 proj_weight=None, mm=None):
    mm = mm or MM_MODE
    wp_np = np.float16 if mm == "f16" else np.float32
    x = np.asarray(x, np.float32)
    gn_weight = np.asarray(gn_weight, np.float32)
    gn_bias = np.asarray(gn_bias, np.float32)
    qkv_weight = np.asarray(qkv_weight, np.float32)
    qkv_bias = np.asarray(qkv_bias, np.float32)
    gmat, gbc = _group_mats(gn_weight)
    gnw = np.ascontiguousarray(gn_weight.reshape(2, P).T)   # [128, 2]
    gnb = np.ascontiguousarray(gn_bias.reshape(2, P).T)

    def wslice(row0):
        w = qkv_weight[row0:row0 + HD, :]            # [64, 256]
        return w.T.reshape(2, P, HD).transpose(1, 0, 2)

    def bias2(off):
        # raw bias + W @ gn_bias (the additive part of the GN affine)
        b = (qkv_bias[off:off + HD]
             + qkv_weight[off:off + HD, :] @ gn_bias).reshape(HD, 1)
        return np.vstack([b, b])

    wps = [None] * NH
    if proj_weight is not None:
        pw = np.asarray(proj_weight, np.float32)
        wps = [np.ascontiguousarray(
            pw[:, h * HD:(h + 1) * HD].T.astype(wp_np)) for h in range(NH)]

    gmat = gmat.copy()
    gmat[:, 1, :] /= HW  # po=1 stats arrive as raw sums
    in_maps = []
    for b in range(B):
        xt = np.ascontiguousarray(
            x[b].reshape(2, P, HW).transpose(1, 0, 2).astype(wp_np))
        for h in range(NH):
            wqkv = np.concatenate(
                [wslice(h * HD), wslice(C + h * HD), wslice(2 * C + h * HD)],
                axis=2).astype(np.float32)
            bv = np.zeros((P, 1), np.float32)
            vrow = 2 * C + h * HD
            bv[0:HD, 0] = (qkv_bias[vrow:vrow + HD]
                           + qkv_weight[vrow:vrow + HD, :] @ gn_bias)
            aux = np.concatenate(
                [bias2(h * HD), bias2(C + h * HD), bv,
                 gnw[:, 0:1], gnw[:, 1:2], gnb[:, 0:1], gnb[:, 1:2],
                 gmat[:, 0, :], gmat[:, 1, :]], axis=1).astype(np.float32)
            in_maps.append({
                "x": xt,
                "wqkv": np.ascontiguousarray(wqkv),
                "wp": wps[h],
                "aux": np.ascontiguousarray(aux),
                "gbc": gbc,
            })
    return in_maps


def combine_outputs(results, x, proj_weight, proj_bias):
    """results: 8 dicts with 'out' [128,2,HW], 'den' [NIB,IB], 'bve' [HD,1]."""
    x = np.asarray(x, np.float32)
    proj_weight = np.asarray(proj_weight, np.float32)
    proj_bias = np.asarray(proj_bias, np.float32)
    y = np.empty((B, C, H, W), np.float32)
    for b in range(B):
        acc = x[b].reshape(C, HW) + proj_bias[:, None]
        for h in range(NH):
            r = results[b * NH + h]
            part = np.asarray(r["out"]).transpose(1, 0, 2).reshape(C, HW)
            den = np.asarray(r["den"]).reshape(HW)
            bve = np.asarray(r["bve"]).reshape(HD)
            ch = proj_weight[:, h * HD:(h + 1) * HD] @ bve
            acc = acc + part / den[None, :] + ch[:, None]
        y[b] = acc.reshape(C, H, W)
    return y


def kernel(x, gn_weight, gn_bias, qkv_weight, qkv_bias, proj_weight, proj_bias):
    from concourse.bass_utils import run_bass_kernel_spmd

    nc = get_module()
    in_maps = make_in_maps(x, gn_weight, gn_bias, qkv_weight, qkv_bias,
                           proj_weight=proj_weight)
    res = run_bass_kernel_spmd(nc, in_maps, core_ids=list(range(NCORES)))
    return combine_outputs(res.results, x, proj_weight, proj_bias)
